# revision 25
# baseline (speedup 1.0000x reference)
"""Trainium2 Bass kernel for nn_BodyAgnosticNACPG (N=4096 coupled oscillators,
fully-connected Gauss-Seidel sweep).

Math: R[i,j] = rot(phase_i - phase_j) = rot(phase_i) @ rot(-phase_j), and the
adjacency is complete-minus-self, so the coupling sum for oscillator i is
    coup_i = (COUP/deg) * rot(phase_i) @ (S_i - u_i),   u_j = rot(-phase_j) @ xy_j
with S_i = sum_j u_j^(current).  Updating i changes S by DT*rot(-phase_i)@dot_i,
so with z_j = DT*G_j dot_j and D_i = sum_{j<i} z_j (exclusive prefix):
    dot_i = clip(q_i + k*P_i @ D_i, lo_i, hi_i)
    q_i   = K_i x_i - k*x_i + k*P_i @ S0      (all precomputable in parallel)
The k = COUP/4095 ~ 2e-5 coupling makes the fixed point contract at ~8e-4 per
sweep, so 2 evaluations (one prefix-sum round) reach the fp32 noise floor.

On-device layout: [128 partition x 32 free], element i -> [i//32, i%32]; the
x/y components of most intermediates are packed side by side in [128, 64]
tiles so one Vector op handles both.  The exclusive prefix sum is a
per-partition tensor_tensor_scan plus one cross-partition carry matmul
(strict-upper-triangular ones, rhs [128,2] = both components); the S0
partition-reduce-and-broadcast is one matmul with an all-ones matrix.

Written in raw Bass (BSP Block + explicit semaphores) because this
toolchain's walrus rejects TileContext's tail drain (its multi-sem-wait CTRL
instruction exceeds the 1-wait ISA slot).  Hardware quirks measured on this
silicon and reflected here:
  * A DVE instruction reading a tensor written by the immediately preceding
    DVE instruction sees stale data (no interlock at distance 1; distance 2
    measured safe).  The Seq helper enforces read-after-write distance >= 3,
    inserting memset spacers when the natural interleave isn't enough.
  * tensor_max (the method) and stt accum_out are broken; tensor_tensor
    (op=max/min) and tensor_reduce are used instead.
  * GpSimd affine_select deadlocks against concurrent DVE work, so the
    triangular/ones matrices ship with the input DMA (second, non-blocking
    transfer) instead of being built on-device.
Engine split: Pool(gpsimd) runs the DMAs; ACT prewarms the Sin table during
the DMA, computes both sines in ONE packed activation (cos(p) = sin(p+pi/2)
folded into the range reduction), and produces the scaled trig copies and
clip bounds off the critical path; PE does one warmup + 2 batched matmuls;
DVE runs the ~70-op main chain.  Each instruction carries at most one
semaphore wait.

The whole problem is ~200KB of data and O(n) flops, so each of the 8 cores
redundantly computes the full answer (no collectives); core 0's output is
returned.  adj_mask is all-ones by construction (deg = n-1 hardcoded) and
never touches the device.
"""

import numpy as np

N = 4096
P = 128
F = 32  # free dim: N = P * F, element i -> [i // F, i % F]
F2 = 2 * F
NPLANES = 9
WIDE = NPLANES * F + 2 * P  # 9 input planes + strict-upper-tri ones + all-ones

ALPHA = 0.45
DT = 0.01
COUP = 0.08
DIFF = 10.0
EPS = 1e-9
K_COUP = float(np.float32(COUP) / np.float32(N - 1))
PI = float(np.pi)

MIN_RAW_DIST = 2  # measured: dist-1 RAW is broken, dist-2 safe

_CACHE = {}


def _build():
    from contextlib import ExitStack
    import concourse.bass as bass
    import concourse.mybir as mybir

    f32 = mybir.dt.float32
    Act = mybir.ActivationFunctionType
    Alu = mybir.AluOpType
    AxX = mybir.AxisListType.X

    nc = bass.Bass("TRN2", debug=False, target_bir_lowering=False)

    d_inp = nc.dram_tensor("inp", [P, WIDE], f32, kind="ExternalInput")
    d_out = nc.dram_tensor("angles", [P, F], f32, kind="ExternalOutput")

    ctx = ExitStack()
    sem = lambda name: ctx.enter_context(nc.semaphore(name))
    sb = lambda name, w=F: ctx.enter_context(nc.sbuf_tensor(name, [P, w], f32))

    dma_s = sem("dma_s")
    v1 = sem("v1")          # DVE: trig args ready
    a_s = sem("a_s")        # ACT: 1 = sines, 2 = all scaled copies/bounds
    v2 = sem("v2")          # DVE: s0 columns ready
    p_s = sem("p_s")        # PE: 1 = s0 matmul, 2 = carry matmul
    v3 = sem("v3")          # DVE: incl scans ready
    v_done = sem("v_done")  # DVE: output ready

    inp = ctx.enter_context(nc.sbuf_tensor("inpt", [P, WIDE], f32))
    # [128,64] packed tiles (x-half | y-half unless noted)
    packs = """targ cs swp kcs dcs ksw dsw sqp P1 P2 uAB lo hi qp A B f dot
        Dp incl""".split()
    T = {n: sb(n, F2) for n in packs}
    for n in """sargA cargA p2 m1s m2s m1c m2c
        r2 asq a n1 negd d1 d1e rd ratio hr zeta rz bt
        t3 t4 t5 t6 vx vy e1 e2 zx zy
        ynew anga ang zeros spacer""".split():
        T[n] = sb(n)
    T["s0cols"] = sb("s0cols", 2)
    T["lastc"] = sb("lastc", 2)

    psum = lambda name, w: ctx.enter_context(nc.psum_tensor(name, [P, w], f32))
    warm = psum("warm", 1)
    cps = psum("cps", 2)    # [S0x + carry_x | S0y + carry_y] per partition

    def plane(i):
        return inp[:, i * F:(i + 1) * F]

    phase = plane(0); amp = plane(1); wfr = plane(2); ha = plane(3)
    bofs = plane(4); x = plane(5); y = plane(6)
    xy_pk = inp[:, 5 * F:7 * F]    # [x|y]
    xdo_pk = inp[:, 7 * F:9 * F]   # [xdx|xdy]
    upT = inp[:, NPLANES * F:NPLANES * F + P]           # U[k,m]=1 iff k<m
    onesM = inp[:, NPLANES * F + P:NPLANES * F + 2 * P]  # all ones

    def L(n):   # left (x) half of a pack
        return T[n][:, 0:F]

    def R(n):   # right (y) half of a pack
        return T[n][:, F:F2]

    class Seq:
        """Emit DVE ops enforcing intra-engine RAW distance >= MIN_RAW_DIST."""

        def __init__(self, v):
            self.v = v
            self.pos = 0
            self.last_w = {}
            self.n_spacers = 0

        def op(self, fn, reads=(), writes=(), inc=None):
            while any(self.pos - self.last_w.get(r, -10) < MIN_RAW_DIST
                      for r in reads):
                self.v.memset(T["spacer"][:, 0:F], 0.0)
                self.pos += 1
                self.n_spacers += 1
            inst = fn()
            if inc is not None:
                inst.then_inc(inc)
            for w in writes:
                self.last_w[w] = self.pos
            self.pos += 1

    with nc.Block(no_gpsimd_drain=True) as block:

        @block.gpsimd
        def _(g):
            NF = NPLANES * F
            g.dma_start(out=inp[:, 0:NF], in_=d_inp[:, 0:NF]).then_inc(dma_s, 16)
            g.dma_start(out=inp[:, NF:WIDE], in_=d_inp[:, NF:WIDE]
                        ).then_inc(dma_s, 16)
            g.wait_ge(v_done, 1)
            g.dma_start(out=d_out[:, :], in_=T["ang"][:, :]).then_inc(dma_s, 16)
            g.wait_ge(dma_s, 48)

        @block.scalar
        def _(act):
            # dummy Sin: pulls the ACT table while the input DMA runs
            act.activation(out=T["lo"][:, 0:1], in_=T["lo"][:, 0:1],
                           func=Act.Sin)
            act.wait_ge(dma_s, 16)
            # clip bounds (Copy with +-DIFF bias), off the DVE critical path
            act.activation(out=T["lo"][:, :], in_=xdo_pk, func=Act.Copy,
                           bias=-DIFF)
            act.activation(out=T["hi"][:, :], in_=xdo_pk, func=Act.Copy,
                           bias=DIFF)
            act.wait_ge(v1, 1)
            # targ = [carg+pi/2 | sarg]  ->  cs = [cos(phase) | sin(phase)]
            act.activation(out=T["cs"][:, :], in_=T["targ"][:, :], func=Act.Sin
                           ).then_inc(a_s)
            # swapped and scaled copies: swp=[s|c], kcs=k*[c|s], dcs=DT*[c|s],
            # ksw=k*[s|c], dsw=DT*[s|c]
            act.activation(out=L("swp"), in_=R("cs"), func=Act.Copy)
            act.activation(out=R("swp"), in_=L("cs"), func=Act.Copy)
            act.activation(out=T["kcs"][:, :], in_=T["cs"][:, :], func=Act.Copy,
                           scale=K_COUP)
            act.activation(out=T["dcs"][:, :], in_=T["cs"][:, :], func=Act.Copy,
                           scale=DT)
            act.activation(out=T["ksw"][:, :], in_=T["swp"][:, :], func=Act.Copy,
                           scale=K_COUP)
            act.activation(out=T["dsw"][:, :], in_=T["swp"][:, :], func=Act.Copy,
                           scale=DT).then_inc(a_s)

        @block.tensor
        def _(pe):
            pe.wait_ge(dma_s, 32)
            pe.matmul(warm[:, :], upT, inp[:, 0:1])
            pe.wait_ge(v2, 1)
            # cps = ones.T @ s0cols  (+)  upT.T @ lastc  ->  [S0 + carry]
            pe.matmul(cps[:, :], onesM, T["s0cols"][:, :], start=True,
                      stop=False)
            pe.wait_ge(v3, 1)
            pe.matmul(cps[:, :], upT, T["lastc"][:, :], start=False, stop=True
                      ).then_inc(p_s)

        @block.vector
        def _(v):
            q = Seq(v)
            t = lambda n: T[n][:, :]

            def TS(out, in0, s1, op0, s2=None, op1=None, reads=(), writes=(),
                   inc=None):
                def emit():
                    if op1 is not None:
                        return v.tensor_scalar(out=out, in0=in0, scalar1=s1,
                                               scalar2=s2, op0=op0, op1=op1)
                    return v.tensor_scalar(out=out, in0=in0, scalar1=s1,
                                           scalar2=s2, op0=op0)
                q.op(emit, reads, writes, inc)

            def STT(out, in0, sc, in1, op0, op1, reads=(), writes=(), inc=None):
                q.op(lambda: v.scalar_tensor_tensor(
                    out=out, in0=in0, scalar=sc, in1=in1, op0=op0, op1=op1),
                    reads, writes, inc)

            def TT(out, in0, in1, op, reads=(), writes=(), inc=None):
                q.op(lambda: v.tensor_tensor(out=out, in0=in0, in1=in1, op=op),
                     reads, writes, inc)

            v.wait_ge(dma_s, 16)
            # --- trig args: sarg=wrap(phase); carg2=wrap(phase+pi/2) ---
            TS(t("p2"), phase, PI / 2, Alu.add, writes=["p2"])
            TS(t("m1s"), phase, PI, Alu.is_gt, writes=["m1s"])
            TS(t("m2s"), phase, -PI, Alu.is_lt, writes=["m2s"])
            TS(t("m1c"), phase, PI / 2, Alu.is_gt, writes=["m1c"])
            TS(t("m2c"), phase, -1.5 * PI, Alu.is_lt, writes=["m2c"])
            STT(t("sargA"), t("m1s"), -2 * PI, phase, Alu.mult, Alu.add,
                reads=["m1s"], writes=["sargA"])
            STT(t("cargA"), t("m1c"), -2 * PI, t("p2"), Alu.mult, Alu.add,
                reads=["m1c", "p2"], writes=["cargA"])
            STT(R("targ"), t("m2s"), 2 * PI, t("sargA"), Alu.mult, Alu.add,
                reads=["m2s", "sargA"], writes=["targ"])
            STT(L("targ"), t("m2c"), 2 * PI, t("cargA"), Alu.mult, Alu.add,
                reads=["m2c", "cargA"], writes=["targ"], inc=v1)

            # --- c/s-independent precompute (overlaps ACT) ---
            TT(t("sqp"), xy_pk, xy_pk, Alu.mult, writes=["sqp"])
            TS(t("negd"), xdo_pk[:, 0:F], -1.0, Alu.mult, writes=["negd"])
            TS(t("n1"), xdo_pk[:, 0:F], EPS, Alu.add, writes=["n1"])
            TT(t("r2"), L("sqp"), R("sqp"), Alu.add, reads=["sqp"],
               writes=["r2"])
            TT(t("d1"), t("negd"), xdo_pk[:, 0:F], Alu.max, reads=["negd"],
               writes=["d1"])
            q.op(lambda: v.memset(t("zeros"), 0.0), writes=["zeros"])
            TT(t("asq"), t("r2"), t("r2"), Alu.mult, reads=["r2"],
               writes=["asq"])
            TS(t("d1e"), t("d1"), EPS, Alu.add, reads=["d1"], writes=["d1e"])
            TS(t("a"), t("asq"), -ALPHA, Alu.mult, ALPHA, Alu.add,
               reads=["asq"], writes=["a"])
            q.op(lambda: v.reciprocal(t("rd"), t("d1e")), reads=["d1e"],
                 writes=["rd"])
            TT(t("t3"), t("a"), x, Alu.mult, reads=["a"], writes=["t3"])
            TT(t("ratio"), t("n1"), t("rd"), Alu.mult, reads=["n1", "rd"],
               writes=["ratio"])
            TT(t("t4"), t("a"), y, Alu.mult, reads=["a"], writes=["t4"])
            TT(t("hr"), ha, t("ratio"), Alu.mult, reads=["ratio"], writes=["hr"])
            TS(t("zeta"), t("hr"), -1.0, Alu.mult, 1.0 + EPS, Alu.add,
               reads=["hr"], writes=["zeta"])
            q.op(lambda: v.reciprocal(t("rz"), t("zeta")), reads=["zeta"],
                 writes=["rz"])
            TT(t("bt"), wfr, t("rz"), Alu.mult, reads=["rz"], writes=["bt"])
            TT(t("t5"), t("bt"), y, Alu.mult, reads=["bt"], writes=["t5"])
            TT(t("t6"), t("bt"), x, Alu.mult, reads=["bt"], writes=["t6"])
            TT(t("vx"), t("t3"), t("t5"), Alu.subtract, reads=["t3", "t5"],
               writes=["vx"])
            TT(t("vy"), t("t6"), t("t4"), Alu.add, reads=["t6", "t4"],
               writes=["vy"])

            # --- e = v - k*xy: iteration-0 dot basis.  The k*P@S0 term is
            # dropped from iteration 0 (its effect on the final output is
            # ~1e-8, below fp32) and S0 is instead folded into the carry
            # matmul, so iteration 0 has NO PE dependency at all. ---
            STT(L("qp"), x, -K_COUP, t("vx"), Alu.mult, Alu.add,
                reads=["vx"], writes=["qp"])
            STT(R("qp"), y, -K_COUP, t("vy"), Alu.mult, Alu.add,
                reads=["vy"], writes=["qp"])
            # dot0/z/scan chain, with the S0 column-sum ops (needed only by
            # the PE matmul) interleaved as the RAW-distance fillers
            v.wait_ge(a_s, 1)
            TT(t("dot"), t("qp"), t("lo"), Alu.max, reads=["qp"],
               writes=["dot"])
            TT(t("P1"), t("cs"), xy_pk, Alu.mult, writes=["P1"])
            TT(t("dot"), t("dot"), t("hi"), Alu.min, reads=["dot"],
               writes=["dot"])
            TS(R("P2"), x, -1.0, Alu.mult, writes=["P2"])
            q.op(lambda: v.tensor_copy(L("P2"), y), writes=["P2"])
            v.wait_ge(a_s, 2)
            TT(t("A"), t("dcs"), t("dot"), Alu.mult, reads=["dot"],
               writes=["A"])
            q.op(lambda: v.tensor_reduce(T["s0cols"][:, 0:1], t("P1"), AxX,
                                         Alu.add),
                 reads=["P1"], writes=["s0cols"])
            TT(t("B"), t("dsw"), t("dot"), Alu.mult, reads=["dot"],
               writes=["B"])
            TT(t("uAB"), t("cs"), t("P2"), Alu.mult, reads=["P2"],
               writes=["uAB"])
            TT(t("zx"), L("A"), R("A"), Alu.add, reads=["A"], writes=["zx"])
            q.op(lambda: v.tensor_reduce(T["s0cols"][:, 1:2], t("uAB"), AxX,
                                         Alu.add),
                 reads=["uAB"], writes=["s0cols"], inc=v2)
            TT(t("zy"), R("B"), L("B"), Alu.subtract, reads=["B"],
               writes=["zy"])
            # per-partition z totals via reduce (not the scan tails) so the
            # PE carry matmul overlaps the scans below
            q.op(lambda: v.tensor_reduce(T["lastc"][:, 0:1], t("zx"), AxX,
                                         Alu.add),
                 reads=["zx"], writes=["lastc"])
            q.op(lambda: v.tensor_reduce(T["lastc"][:, 1:2], t("zy"), AxX,
                                         Alu.add),
                 reads=["zy"], writes=["lastc"], inc=v3)
            q.op(lambda: v.tensor_tensor_scan(
                out=L("incl"), data0=t("zx"), data1=t("zeros"), initial=0.0,
                op0=Alu.add, op1=Alu.add),
                reads=["zx", "zeros"], writes=["incl"])
            q.op(lambda: v.tensor_tensor_scan(
                out=R("incl"), data0=t("zy"), data1=t("zeros"), initial=0.0,
                op0=Alu.add, op1=Alu.add),
                reads=["zy", "zeros"], writes=["incl"])

            # --- D+S0 = excl prefix + S0 (single accumulated PE psum) ---
            v.wait_ge(p_s, 1)
            STT(L("Dp"), L("incl"), cps[:, 0:1], t("zx"), Alu.add,
                Alu.subtract, reads=["incl", "zx"], writes=["Dp"])
            STT(R("Dp"), R("incl"), cps[:, 1:2], t("zy"), Alu.add,
                Alu.subtract, reads=["incl", "zy"], writes=["Dp"])
            TT(t("A"), t("kcs"), t("Dp"), Alu.mult, reads=["Dp"], writes=["A"])
            TT(t("B"), t("ksw"), t("Dp"), Alu.mult, reads=["Dp"], writes=["B"])
            TT(L("f"), L("A"), R("A"), Alu.subtract, reads=["A"], writes=["f"])
            TT(R("f"), L("B"), R("B"), Alu.add, reads=["B"], writes=["f"])
            TT(t("f"), t("f"), t("qp"), Alu.add, reads=["f", "qp"],
               writes=["f"])
            TT(t("dot"), t("f"), t("lo"), Alu.max, reads=["f"], writes=["dot"])
            TT(t("dot"), t("dot"), t("hi"), Alu.min, reads=["dot"],
               writes=["dot"])
            # angles = amp * (y + DT*doty) + b
            STT(t("ynew"), R("dot"), DT, y, Alu.mult, Alu.add,
                reads=["dot"], writes=["ynew"])
            TT(t("anga"), amp, t("ynew"), Alu.mult, reads=["ynew"],
               writes=["anga"])
            TT(t("ang"), t("anga"), bofs, Alu.add, reads=["anga"],
               writes=["ang"], inc=v_done)

    ctx.close()
    _strip_init_barrier(nc)
    return nc


def _strip_init_barrier(nc):
    """Remove the Bass-init all-engine rendezvous (4 Drains + EVSEM butterfly,
    ~3us) from the entry block.  Every ordering this kernel needs flows through
    its explicit semaphores: the Pool const-memsets precede the input DMA in
    Pool program order and all other engines gate on dma_s, so the rendezvous
    is redundant.  The Block-exit barrier is left untouched (removing it was
    observed to race)."""
    bb = nc.main_func.blocks[0]
    keep = [ins for ins in bb.instructions
            if not (type(ins).__name__ == "InstDrain"
                    or (type(ins).__name__ == "InstEventSemaphore"
                        and "barrier" in ins.name))]
    if len(keep) != len(bb.instructions):
        del bb.instructions[:]
        for ins in keep:
            bb.instructions.append(ins)


def _get_nc():
    if "nc" not in _CACHE:
        _CACHE["nc"] = _build()
    return _CACHE["nc"]


def pack_inputs(phase, amplitudes, w, ha, b, xy, xy_dot_old):
    f = np.float32
    xy = np.asarray(xy, f)
    xdo = np.asarray(xy_dot_old, f)
    planes = [
        np.asarray(phase, f).reshape(P, F),
        np.asarray(amplitudes, f).reshape(P, F),
        np.asarray(w, f).reshape(P, F),
        np.asarray(ha, f).reshape(P, F),
        np.asarray(b, f).reshape(P, F),
        np.ascontiguousarray(xy[:, 0]).reshape(P, F),
        np.ascontiguousarray(xy[:, 1]).reshape(P, F),
        np.ascontiguousarray(xdo[:, 0]).reshape(P, F),
        np.ascontiguousarray(xdo[:, 1]).reshape(P, F),
        np.triu(np.ones((P, P), f), k=1),
        np.ones((P, P), f),
    ]
    return {"inp": np.ascontiguousarray(np.concatenate(planes, axis=1))}


def kernel(phase, amplitudes, w, ha, b, xy, xy_dot_old, adj_mask):
    from concourse.bass_utils import run_bass_kernel_spmd

    nc = _get_nc()
    in_map = pack_inputs(phase, amplitudes, w, ha, b, xy, xy_dot_old)
    n_cores = 8
    res = run_bass_kernel_spmd(nc, [in_map] * n_cores, core_ids=list(range(n_cores)))
    return np.asarray(res.results[0]["angles"], dtype=np.float32).reshape(N)


# revision 28
# speedup vs baseline: 1.1760x; 1.1760x over previous
"""Trainium2 Bass kernel for nn_BodyAgnosticNACPG (N=4096 coupled oscillators,
fully-connected Gauss-Seidel sweep).

Math: R[i,j] = rot(phase_i - phase_j) = rot(phase_i) @ rot(-phase_j), and the
adjacency is complete-minus-self, so the coupling sum for oscillator i is
    coup_i = (COUP/deg) * rot(phase_i) @ (S_i - u_i),   u_j = rot(-phase_j) @ xy_j
with S_i = sum_j u_j^(current).  Updating i changes S by DT*rot(-phase_i)@dot_i,
so with z_j = DT*G_j dot_j and D_i = sum_{j<i} z_j (exclusive prefix):
    dot_i = clip(q_i + k*P_i @ D_i, lo_i, hi_i)
    q_i   = K_i x_i - k*x_i + k*P_i @ S0      (all precomputable in parallel)
The k = COUP/4095 ~ 2e-5 coupling makes the fixed point contract at ~8e-4 per
sweep, so 2 evaluations (one prefix-sum round) reach the fp32 noise floor.

On-device layout: [128 partition x 32 free], element i -> [i//32, i%32]; the
x/y components of most intermediates are packed side by side in [128, 64]
tiles so one Vector op handles both.  The exclusive prefix sum is a
per-partition tensor_tensor_scan plus one cross-partition carry matmul
(strict-upper-triangular ones, rhs [128,2] = both components); the S0
partition-reduce-and-broadcast is one matmul with an all-ones matrix.

Written in raw Bass (BSP Block + explicit semaphores) because this
toolchain's walrus rejects TileContext's tail drain (its multi-sem-wait CTRL
instruction exceeds the 1-wait ISA slot).  Hardware quirks measured on this
silicon and reflected here:
  * A DVE instruction reading a tensor written by the immediately preceding
    DVE instruction sees stale data (no interlock at distance 1; distance 2
    measured safe).  The Seq helper enforces read-after-write distance >= 3,
    inserting memset spacers when the natural interleave isn't enough.
  * tensor_max (the method) and stt accum_out are broken; tensor_tensor
    (op=max/min) and tensor_reduce are used instead.
  * GpSimd affine_select deadlocks against concurrent DVE work, so the
    triangular/ones matrices ship with the input DMA (second, non-blocking
    transfer) instead of being built on-device.
Engine split: Pool(gpsimd) runs the DMAs; ACT prewarms the Sin table during
the DMA, computes both sines in ONE packed activation (cos(p) = sin(p+pi/2)
folded into the range reduction), and produces the scaled trig copies and
clip bounds off the critical path; PE does one warmup + 2 batched matmuls;
DVE runs the ~70-op main chain.  Each instruction carries at most one
semaphore wait.

The whole problem is ~200KB of data and O(n) flops, so each of the 8 cores
redundantly computes the full answer (no collectives); core 0's output is
returned.  adj_mask is all-ones by construction (deg = n-1 hardcoded) and
never touches the device.
"""

import numpy as np

N = 4096
P = 128
F = 32  # free dim: N = P * F, element i -> [i // F, i % F]
F2 = 2 * F
NPLANES = 9
WIDE = NPLANES * F + 2 * P  # 9 input planes + strict-upper-tri ones + all-ones

ALPHA = 0.45
DT = 0.01
COUP = 0.08
DIFF = 10.0
EPS = 1e-9
K_COUP = float(np.float32(COUP) / np.float32(N - 1))
PI = float(np.pi)

MIN_RAW_DIST = 2  # measured: dist-1 RAW is broken, dist-2 safe

_CACHE = {}


def _build():
    from contextlib import ExitStack
    import concourse.bass as bass
    import concourse.mybir as mybir

    f32 = mybir.dt.float32
    Act = mybir.ActivationFunctionType
    Alu = mybir.AluOpType
    AxX = mybir.AxisListType.X

    nc = bass.Bass("TRN2", debug=False, target_bir_lowering=False)

    d_inp = nc.dram_tensor("inp", [P, WIDE], f32, kind="ExternalInput")
    d_out = nc.dram_tensor("angles", [P, F], f32, kind="ExternalOutput")

    ctx = ExitStack()
    sem = lambda name: ctx.enter_context(nc.semaphore(name))
    sb = lambda name, w=F: ctx.enter_context(nc.sbuf_tensor(name, [P, w], f32))

    dma_s = sem("dma_s")
    v1 = sem("v1")          # DVE: trig args ready
    a_s = sem("a_s")        # ACT: 1 = sines, 2 = all scaled copies/bounds
    v2 = sem("v2")          # DVE: s0 columns ready
    p_s = sem("p_s")        # PE: 1 = s0 matmul, 2 = carry matmul
    v3 = sem("v3")          # DVE: incl scans ready
    v_done = sem("v_done")  # DVE: output ready

    inp = ctx.enter_context(nc.sbuf_tensor("inpt", [P, WIDE], f32))
    # [128,64] packed tiles (x-half | y-half unless noted)
    packs = """targ cs swp kcs dcs ksw dsw sqp P1 P2 uAB lo hi qp A B f dot
        Dp incl""".split()
    T = {n: sb(n, F2) for n in packs}
    for n in """sargA cargA p2 m1s m2s m1c m2c
        r2 asq a n1 negd d1 d1e rd ratio hr zeta rz bt
        t3 t4 t5 t6 vx vy e1 e2 zx zy
        ynew anga ang zeros spacer""".split():
        T[n] = sb(n)
    T["s0cols"] = sb("s0cols", 2)
    T["lastc"] = sb("lastc", 2)

    psum = lambda name, w: ctx.enter_context(nc.psum_tensor(name, [P, w], f32))
    warm = psum("warm", 1)
    cps = psum("cps", 2)    # [S0x + carry_x | S0y + carry_y] per partition

    def plane(i):
        return inp[:, i * F:(i + 1) * F]

    phase = plane(0); amp = plane(1); wfr = plane(2); ha = plane(3)
    bofs = plane(4); x = plane(5); y = plane(6)
    xy_pk = inp[:, 5 * F:7 * F]    # [x|y]
    xdo_pk = inp[:, 7 * F:9 * F]   # [xdx|xdy]
    upT = inp[:, NPLANES * F:NPLANES * F + P]           # U[k,m]=1 iff k<m
    onesM = inp[:, NPLANES * F + P:NPLANES * F + 2 * P]  # all ones

    def L(n):   # left (x) half of a pack
        return T[n][:, 0:F]

    def R(n):   # right (y) half of a pack
        return T[n][:, F:F2]

    class Seq:
        """Emit DVE ops enforcing intra-engine RAW distance >= MIN_RAW_DIST."""

        def __init__(self, v):
            self.v = v
            self.pos = 0
            self.last_w = {}
            self.n_spacers = 0

        def op(self, fn, reads=(), writes=(), inc=None):
            while any(self.pos - self.last_w.get(r, -10) < MIN_RAW_DIST
                      for r in reads):
                self.v.memset(T["spacer"][:, 0:F], 0.0)
                self.pos += 1
                self.n_spacers += 1
            inst = fn()
            if inc is not None:
                inst.then_inc(inc)
            for w in writes:
                self.last_w[w] = self.pos
            self.pos += 1

    with nc.Block(no_gpsimd_drain=True) as block:

        @block.gpsimd
        def _(g):
            NF = NPLANES * F
            # phase plane first: unblocks the DVE wrap + ACT Sin early
            g.dma_start(out=inp[:, 0:F], in_=d_inp[:, 0:F]).then_inc(dma_s, 16)
            g.dma_start(out=inp[:, F:WIDE], in_=d_inp[:, F:WIDE]
                        ).then_inc(dma_s, 16)
            g.wait_ge(v_done, 1)
            g.dma_start(out=d_out[:, :], in_=T["ang"][:, :]).then_inc(dma_s, 16)
            g.wait_ge(dma_s, 48)

        @block.scalar
        def _(act):
            # dummy Sin: pulls the ACT table while the input DMA runs
            act.activation(out=T["lo"][:, 0:1], in_=T["lo"][:, 0:1],
                           func=Act.Sin)
            act.wait_ge(dma_s, 32)
            # clip bounds (Copy with +-DIFF bias), off the DVE critical path
            act.activation(out=T["lo"][:, :], in_=xdo_pk, func=Act.Copy,
                           bias=-DIFF)
            act.activation(out=T["hi"][:, :], in_=xdo_pk, func=Act.Copy,
                           bias=DIFF)
            act.wait_ge(v1, 1)
            # targ = [carg+pi/2 | sarg]  ->  cs = [cos(phase) | sin(phase)]
            act.activation(out=T["cs"][:, :], in_=T["targ"][:, :], func=Act.Sin
                           ).then_inc(a_s)
            # swapped and scaled copies: swp=[s|c], kcs=k*[c|s], dcs=DT*[c|s],
            # ksw=k*[s|c], dsw=DT*[s|c]
            act.activation(out=L("swp"), in_=R("cs"), func=Act.Copy)
            act.activation(out=R("swp"), in_=L("cs"), func=Act.Copy)
            act.activation(out=T["kcs"][:, :], in_=T["cs"][:, :], func=Act.Copy,
                           scale=K_COUP)
            act.activation(out=T["dcs"][:, :], in_=T["cs"][:, :], func=Act.Copy,
                           scale=DT)
            act.activation(out=T["ksw"][:, :], in_=T["swp"][:, :], func=Act.Copy,
                           scale=K_COUP)
            act.activation(out=T["dsw"][:, :], in_=T["swp"][:, :], func=Act.Copy,
                           scale=DT).then_inc(a_s)

        @block.tensor
        def _(pe):
            pe.wait_ge(dma_s, 32)
            pe.matmul(warm[:, :], upT, inp[:, 0:1])
            pe.wait_ge(v2, 1)
            # cps = ones.T @ s0cols  (+)  upT.T @ lastc  ->  [S0 + carry]
            pe.matmul(cps[:, :], onesM, T["s0cols"][:, :], start=True,
                      stop=False)
            pe.wait_ge(v3, 1)
            pe.matmul(cps[:, :], upT, T["lastc"][:, :], start=False, stop=True
                      ).then_inc(p_s)

        @block.vector
        def _(v):
            q = Seq(v)
            t = lambda n: T[n][:, :]

            def TS(out, in0, s1, op0, s2=None, op1=None, reads=(), writes=(),
                   inc=None):
                def emit():
                    if op1 is not None:
                        return v.tensor_scalar(out=out, in0=in0, scalar1=s1,
                                               scalar2=s2, op0=op0, op1=op1)
                    return v.tensor_scalar(out=out, in0=in0, scalar1=s1,
                                           scalar2=s2, op0=op0)
                q.op(emit, reads, writes, inc)

            def STT(out, in0, sc, in1, op0, op1, reads=(), writes=(), inc=None):
                q.op(lambda: v.scalar_tensor_tensor(
                    out=out, in0=in0, scalar=sc, in1=in1, op0=op0, op1=op1),
                    reads, writes, inc)

            def TT(out, in0, in1, op, reads=(), writes=(), inc=None):
                q.op(lambda: v.tensor_tensor(out=out, in0=in0, in1=in1, op=op),
                     reads, writes, inc)

            v.wait_ge(dma_s, 16)
            # --- trig args: sarg=wrap(phase); carg2=wrap(phase+pi/2) ---
            TS(t("p2"), phase, PI / 2, Alu.add, writes=["p2"])
            TS(t("m1s"), phase, PI, Alu.is_gt, writes=["m1s"])
            TS(t("m2s"), phase, -PI, Alu.is_lt, writes=["m2s"])
            TS(t("m1c"), phase, PI / 2, Alu.is_gt, writes=["m1c"])
            TS(t("m2c"), phase, -1.5 * PI, Alu.is_lt, writes=["m2c"])
            STT(t("sargA"), t("m1s"), -2 * PI, phase, Alu.mult, Alu.add,
                reads=["m1s"], writes=["sargA"])
            STT(t("cargA"), t("m1c"), -2 * PI, t("p2"), Alu.mult, Alu.add,
                reads=["m1c", "p2"], writes=["cargA"])
            STT(R("targ"), t("m2s"), 2 * PI, t("sargA"), Alu.mult, Alu.add,
                reads=["m2s", "sargA"], writes=["targ"])
            STT(L("targ"), t("m2c"), 2 * PI, t("cargA"), Alu.mult, Alu.add,
                reads=["m2c", "cargA"], writes=["targ"], inc=v1)

            # --- c/s-independent precompute (overlaps ACT) ---
            v.wait_ge(dma_s, 32)
            TT(t("sqp"), xy_pk, xy_pk, Alu.mult, writes=["sqp"])
            TS(t("negd"), xdo_pk[:, 0:F], -1.0, Alu.mult, writes=["negd"])
            TS(t("n1"), xdo_pk[:, 0:F], EPS, Alu.add, writes=["n1"])
            TT(t("r2"), L("sqp"), R("sqp"), Alu.add, reads=["sqp"],
               writes=["r2"])
            TT(t("d1"), t("negd"), xdo_pk[:, 0:F], Alu.max, reads=["negd"],
               writes=["d1"])
            q.op(lambda: v.memset(t("zeros"), 0.0), writes=["zeros"])
            TT(t("asq"), t("r2"), t("r2"), Alu.mult, reads=["r2"],
               writes=["asq"])
            TS(t("d1e"), t("d1"), EPS, Alu.add, reads=["d1"], writes=["d1e"])
            TS(t("a"), t("asq"), -ALPHA, Alu.mult, ALPHA, Alu.add,
               reads=["asq"], writes=["a"])
            q.op(lambda: v.reciprocal(t("rd"), t("d1e")), reads=["d1e"],
                 writes=["rd"])
            TT(t("t3"), t("a"), x, Alu.mult, reads=["a"], writes=["t3"])
            TT(t("ratio"), t("n1"), t("rd"), Alu.mult, reads=["n1", "rd"],
               writes=["ratio"])
            TT(t("t4"), t("a"), y, Alu.mult, reads=["a"], writes=["t4"])
            TT(t("hr"), ha, t("ratio"), Alu.mult, reads=["ratio"], writes=["hr"])
            TS(t("zeta"), t("hr"), -1.0, Alu.mult, 1.0 + EPS, Alu.add,
               reads=["hr"], writes=["zeta"])
            q.op(lambda: v.reciprocal(t("rz"), t("zeta")), reads=["zeta"],
                 writes=["rz"])
            TT(t("bt"), wfr, t("rz"), Alu.mult, reads=["rz"], writes=["bt"])
            TT(t("t5"), t("bt"), y, Alu.mult, reads=["bt"], writes=["t5"])
            TT(t("t6"), t("bt"), x, Alu.mult, reads=["bt"], writes=["t6"])
            TT(t("vx"), t("t3"), t("t5"), Alu.subtract, reads=["t3", "t5"],
               writes=["vx"])
            TT(t("vy"), t("t6"), t("t4"), Alu.add, reads=["t6", "t4"],
               writes=["vy"])

            # --- e = v - k*xy: iteration-0 dot basis.  The k*P@S0 term is
            # dropped from iteration 0 (its effect on the final output is
            # ~1e-8, below fp32) and S0 is instead folded into the carry
            # matmul, so iteration 0 has NO PE dependency at all. ---
            STT(L("qp"), x, -K_COUP, t("vx"), Alu.mult, Alu.add,
                reads=["vx"], writes=["qp"])
            STT(R("qp"), y, -K_COUP, t("vy"), Alu.mult, Alu.add,
                reads=["vy"], writes=["qp"])
            # dot0/z/scan chain, with the S0 column-sum ops (needed only by
            # the PE matmul) interleaved as the RAW-distance fillers
            v.wait_ge(a_s, 1)
            TT(t("dot"), t("qp"), t("lo"), Alu.max, reads=["qp"],
               writes=["dot"])
            TT(t("P1"), t("cs"), xy_pk, Alu.mult, writes=["P1"])
            TT(t("dot"), t("dot"), t("hi"), Alu.min, reads=["dot"],
               writes=["dot"])
            TS(R("P2"), x, -1.0, Alu.mult, writes=["P2"])
            q.op(lambda: v.tensor_copy(L("P2"), y), writes=["P2"])
            v.wait_ge(a_s, 2)
            TT(t("A"), t("dcs"), t("dot"), Alu.mult, reads=["dot"],
               writes=["A"])
            q.op(lambda: v.tensor_reduce(T["s0cols"][:, 0:1], t("P1"), AxX,
                                         Alu.add),
                 reads=["P1"], writes=["s0cols"])
            TT(t("B"), t("dsw"), t("dot"), Alu.mult, reads=["dot"],
               writes=["B"])
            TT(t("uAB"), t("cs"), t("P2"), Alu.mult, reads=["P2"],
               writes=["uAB"])
            TT(t("zx"), L("A"), R("A"), Alu.add, reads=["A"], writes=["zx"])
            q.op(lambda: v.tensor_reduce(T["s0cols"][:, 1:2], t("uAB"), AxX,
                                         Alu.add),
                 reads=["uAB"], writes=["s0cols"], inc=v2)
            TT(t("zy"), R("B"), L("B"), Alu.subtract, reads=["B"],
               writes=["zy"])
            # per-partition z totals via reduce (not the scan tails) so the
            # PE carry matmul overlaps the scans below
            q.op(lambda: v.tensor_reduce(T["lastc"][:, 0:1], t("zx"), AxX,
                                         Alu.add),
                 reads=["zx"], writes=["lastc"])
            q.op(lambda: v.tensor_reduce(T["lastc"][:, 1:2], t("zy"), AxX,
                                         Alu.add),
                 reads=["zy"], writes=["lastc"], inc=v3)
            q.op(lambda: v.tensor_tensor_scan(
                out=L("incl"), data0=t("zx"), data1=t("zeros"), initial=0.0,
                op0=Alu.add, op1=Alu.add),
                reads=["zx", "zeros"], writes=["incl"])
            q.op(lambda: v.tensor_tensor_scan(
                out=R("incl"), data0=t("zy"), data1=t("zeros"), initial=0.0,
                op0=Alu.add, op1=Alu.add),
                reads=["zy", "zeros"], writes=["incl"])

            # --- D+S0 = excl prefix + S0 (single accumulated PE psum) ---
            v.wait_ge(p_s, 1)
            STT(L("Dp"), L("incl"), cps[:, 0:1], t("zx"), Alu.add,
                Alu.subtract, reads=["incl", "zx"], writes=["Dp"])
            STT(R("Dp"), R("incl"), cps[:, 1:2], t("zy"), Alu.add,
                Alu.subtract, reads=["incl", "zy"], writes=["Dp"])
            TT(t("A"), t("kcs"), t("Dp"), Alu.mult, reads=["Dp"], writes=["A"])
            TT(t("B"), t("ksw"), t("Dp"), Alu.mult, reads=["Dp"], writes=["B"])
            TT(L("f"), L("A"), R("A"), Alu.subtract, reads=["A"], writes=["f"])
            TT(R("f"), L("B"), R("B"), Alu.add, reads=["B"], writes=["f"])
            TT(t("f"), t("f"), t("qp"), Alu.add, reads=["f", "qp"],
               writes=["f"])
            TT(t("dot"), t("f"), t("lo"), Alu.max, reads=["f"], writes=["dot"])
            TT(t("dot"), t("dot"), t("hi"), Alu.min, reads=["dot"],
               writes=["dot"])
            # angles = amp * (y + DT*doty) + b
            STT(t("ynew"), R("dot"), DT, y, Alu.mult, Alu.add,
                reads=["dot"], writes=["ynew"])
            TT(t("anga"), amp, t("ynew"), Alu.mult, reads=["ynew"],
               writes=["anga"])
            TT(t("ang"), t("anga"), bofs, Alu.add, reads=["anga"],
               writes=["ang"], inc=v_done)

    ctx.close()
    _strip_init_barrier(nc)
    return nc


def _strip_init_barrier(nc):
    """Remove the Bass-init all-engine rendezvous (4 Drains + EVSEM butterfly,
    ~3us) from the entry block.  Every ordering this kernel needs flows through
    its explicit semaphores: the Pool const-memsets precede the input DMA in
    Pool program order and all other engines gate on dma_s, so the rendezvous
    is redundant.  The Block-exit barrier is left untouched (removing it was
    observed to race)."""
    bb = nc.main_func.blocks[0]
    keep = [ins for ins in bb.instructions
            if not (type(ins).__name__ == "InstDrain"
                    or (type(ins).__name__ == "InstEventSemaphore"
                        and "barrier" in ins.name))]
    if len(keep) != len(bb.instructions):
        del bb.instructions[:]
        for ins in keep:
            bb.instructions.append(ins)


def _get_nc():
    if "nc" not in _CACHE:
        _CACHE["nc"] = _build()
    return _CACHE["nc"]


def pack_inputs(phase, amplitudes, w, ha, b, xy, xy_dot_old):
    f = np.float32
    xy = np.asarray(xy, f)
    xdo = np.asarray(xy_dot_old, f)
    planes = [
        np.asarray(phase, f).reshape(P, F),
        np.asarray(amplitudes, f).reshape(P, F),
        np.asarray(w, f).reshape(P, F),
        np.asarray(ha, f).reshape(P, F),
        np.asarray(b, f).reshape(P, F),
        np.ascontiguousarray(xy[:, 0]).reshape(P, F),
        np.ascontiguousarray(xy[:, 1]).reshape(P, F),
        np.ascontiguousarray(xdo[:, 0]).reshape(P, F),
        np.ascontiguousarray(xdo[:, 1]).reshape(P, F),
        np.triu(np.ones((P, P), f), k=1),
        np.ones((P, P), f),
    ]
    return {"inp": np.ascontiguousarray(np.concatenate(planes, axis=1))}


def kernel(phase, amplitudes, w, ha, b, xy, xy_dot_old, adj_mask):
    from concourse.bass_utils import run_bass_kernel_spmd

    nc = _get_nc()
    in_map = pack_inputs(phase, amplitudes, w, ha, b, xy, xy_dot_old)
    n_cores = 8
    res = run_bass_kernel_spmd(nc, [in_map] * n_cores, core_ids=list(range(n_cores)))
    return np.asarray(res.results[0]["angles"], dtype=np.float32).reshape(N)


# revision 30
# speedup vs baseline: 1.1879x; 1.0101x over previous
"""Trainium2 Bass kernel for nn_BodyAgnosticNACPG (N=4096 coupled oscillators,
fully-connected Gauss-Seidel sweep).

Math: R[i,j] = rot(phase_i - phase_j) = rot(phase_i) @ rot(-phase_j), and the
adjacency is complete-minus-self, so the coupling sum for oscillator i is
    coup_i = (COUP/deg) * rot(phase_i) @ (S_i - u_i),   u_j = rot(-phase_j) @ xy_j
with S_i = sum_j u_j^(current).  Updating i changes S by DT*rot(-phase_i)@dot_i,
so with z_j = DT*G_j dot_j and D_i = sum_{j<i} z_j (exclusive prefix):
    dot_i = clip(q_i + k*P_i @ D_i, lo_i, hi_i)
    q_i   = K_i x_i - k*x_i + k*P_i @ S0      (all precomputable in parallel)
The k = COUP/4095 ~ 2e-5 coupling makes the fixed point contract at ~8e-4 per
sweep, so 2 evaluations (one prefix-sum round) reach the fp32 noise floor.

On-device layout: [128 partition x 32 free], element i -> [i//32, i%32]; the
x/y components of most intermediates are packed side by side in [128, 64]
tiles so one Vector op handles both.  The exclusive prefix sum is a
per-partition tensor_tensor_scan plus one cross-partition carry matmul
(strict-upper-triangular ones, rhs [128,2] = both components); the S0
partition-reduce-and-broadcast is one matmul with an all-ones matrix.

Written in raw Bass (BSP Block + explicit semaphores) because this
toolchain's walrus rejects TileContext's tail drain (its multi-sem-wait CTRL
instruction exceeds the 1-wait ISA slot).  Hardware quirks measured on this
silicon and reflected here:
  * A DVE instruction reading a tensor written by the immediately preceding
    DVE instruction sees stale data (no interlock at distance 1; distance 2
    measured safe).  The Seq helper enforces read-after-write distance >= 3,
    inserting memset spacers when the natural interleave isn't enough.
  * tensor_max (the method) and stt accum_out are broken; tensor_tensor
    (op=max/min) and tensor_reduce are used instead.
  * GpSimd affine_select deadlocks against concurrent DVE work, so the
    triangular/ones matrices ship with the input DMA (second, non-blocking
    transfer) instead of being built on-device.
Engine split: Pool(gpsimd) runs the DMAs; ACT prewarms the Sin table during
the DMA, computes both sines in ONE packed activation (cos(p) = sin(p+pi/2)
folded into the range reduction), and produces the scaled trig copies and
clip bounds off the critical path; PE does one warmup + 2 batched matmuls;
DVE runs the ~70-op main chain.  Each instruction carries at most one
semaphore wait.

The whole problem is ~200KB of data and O(n) flops, so each of the 8 cores
redundantly computes the full answer (no collectives); core 0's output is
returned.  adj_mask is all-ones by construction (deg = n-1 hardcoded) and
never touches the device.
"""

import numpy as np

N = 4096
P = 128
F = 32  # free dim: N = P * F, element i -> [i // F, i % F]
F2 = 2 * F
NPLANES = 9
WIDE = NPLANES * F + 2 * P  # 9 input planes + strict-upper-tri ones + all-ones

ALPHA = 0.45
DT = 0.01
COUP = 0.08
DIFF = 10.0
EPS = 1e-9
K_COUP = float(np.float32(COUP) / np.float32(N - 1))
PI = float(np.pi)

MIN_RAW_DIST = 2  # measured: dist-1 RAW is broken, dist-2 safe

_CACHE = {}


def _build():
    from contextlib import ExitStack
    import concourse.bass as bass
    import concourse.mybir as mybir

    f32 = mybir.dt.float32
    Act = mybir.ActivationFunctionType
    Alu = mybir.AluOpType
    AxX = mybir.AxisListType.X

    nc = bass.Bass("TRN2", debug=False, target_bir_lowering=False)

    d_inp = nc.dram_tensor("inp", [P, WIDE], f32, kind="ExternalInput")
    d_out = nc.dram_tensor("angles", [P, F], f32, kind="ExternalOutput")

    ctx = ExitStack()
    sem = lambda name: ctx.enter_context(nc.semaphore(name))
    sb = lambda name, w=F: ctx.enter_context(nc.sbuf_tensor(name, [P, w], f32))

    dma_s = sem("dma_s")
    dma_b = sem("dma_b")    # planes 1-8
    dma_c = sem("dma_c")    # matrices
    v1 = sem("v1")          # DVE: trig args ready
    a_s = sem("a_s")        # ACT: 1 = sines, 2 = all scaled copies/bounds
    v2 = sem("v2")          # DVE: s0 columns ready
    p_s = sem("p_s")        # PE: 1 = s0 matmul, 2 = carry matmul
    v3 = sem("v3")          # DVE: incl scans ready
    v_done = sem("v_done")  # DVE: output ready

    inp = ctx.enter_context(nc.sbuf_tensor("inpt", [P, WIDE], f32))
    # [128,64] packed tiles (x-half | y-half unless noted)
    packs = """targ cs swp kcs dcs ksw dsw sqp P1 P2 uAB lo hi qp A B f dot
        Dp incl""".split()
    T = {n: sb(n, F2) for n in packs}
    for n in """sargA cargA p2 m1s m2s m1c m2c
        r2 asq a n1 negd d1 d1e rd ratio hr zeta rz bt
        t3 t4 t5 t6 vx vy e1 e2 zx zy
        ynew anga ang zeros spacer""".split():
        T[n] = sb(n)
    T["s0cols"] = sb("s0cols", 2)
    T["lastc"] = sb("lastc", 2)

    psum = lambda name, w: ctx.enter_context(nc.psum_tensor(name, [P, w], f32))
    warm = psum("warm", 1)
    cps = psum("cps", 2)    # [S0x + carry_x | S0y + carry_y] per partition

    def plane(i):
        return inp[:, i * F:(i + 1) * F]

    phase = plane(0); amp = plane(1); wfr = plane(2); ha = plane(3)
    bofs = plane(4); x = plane(5); y = plane(6)
    xy_pk = inp[:, 5 * F:7 * F]    # [x|y]
    xdo_pk = inp[:, 7 * F:9 * F]   # [xdx|xdy]
    upT = inp[:, NPLANES * F:NPLANES * F + P]           # U[k,m]=1 iff k<m
    onesM = inp[:, NPLANES * F + P:NPLANES * F + 2 * P]  # all ones

    def L(n):   # left (x) half of a pack
        return T[n][:, 0:F]

    def R(n):   # right (y) half of a pack
        return T[n][:, F:F2]

    class Seq:
        """Emit DVE ops enforcing intra-engine RAW distance >= MIN_RAW_DIST."""

        def __init__(self, v):
            self.v = v
            self.pos = 0
            self.last_w = {}
            self.n_spacers = 0

        def op(self, fn, reads=(), writes=(), inc=None):
            while any(self.pos - self.last_w.get(r, -10) < MIN_RAW_DIST
                      for r in reads):
                self.v.memset(T["spacer"][:, 0:F], 0.0)
                self.pos += 1
                self.n_spacers += 1
            inst = fn()
            if inc is not None:
                inst.then_inc(inc)
            for w in writes:
                self.last_w[w] = self.pos
            self.pos += 1

    with nc.Block(no_gpsimd_drain=True) as block:

        @block.gpsimd
        def _(g):
            NF = NPLANES * F
            # phase plane first: unblocks the DVE wrap + ACT Sin early
            g.dma_start(out=inp[:, 0:F], in_=d_inp[:, 0:F]).then_inc(dma_s, 16)
            g.dma_start(out=inp[:, F:NF], in_=d_inp[:, F:NF]).then_inc(dma_b, 16)
            g.dma_start(out=inp[:, NF:WIDE], in_=d_inp[:, NF:WIDE]
                        ).then_inc(dma_c, 16)
            g.wait_ge(v_done, 1)
            g.dma_start(out=d_out[:, :], in_=T["ang"][:, :]).then_inc(dma_s, 32)
            g.wait_ge(dma_s, 48)

        @block.scalar
        def _(act):
            # dummy Sin: pulls the ACT table while the input DMA runs
            act.activation(out=T["lo"][:, 0:1], in_=T["lo"][:, 0:1],
                           func=Act.Sin)
            act.wait_ge(dma_b, 16)
            # clip bounds (Copy with +-DIFF bias), off the DVE critical path
            act.activation(out=T["lo"][:, :], in_=xdo_pk, func=Act.Copy,
                           bias=-DIFF)
            act.activation(out=T["hi"][:, :], in_=xdo_pk, func=Act.Copy,
                           bias=DIFF)
            act.wait_ge(v1, 1)
            # targ = [carg+pi/2 | sarg]  ->  cs = [cos(phase) | sin(phase)]
            act.activation(out=T["cs"][:, :], in_=T["targ"][:, :], func=Act.Sin
                           ).then_inc(a_s)
            # swapped and scaled copies: swp=[s|c], kcs=k*[c|s], dcs=DT*[c|s],
            # ksw=k*[s|c], dsw=DT*[s|c]
            act.activation(out=L("swp"), in_=R("cs"), func=Act.Copy)
            act.activation(out=R("swp"), in_=L("cs"), func=Act.Copy)
            act.activation(out=T["kcs"][:, :], in_=T["cs"][:, :], func=Act.Copy,
                           scale=K_COUP)
            act.activation(out=T["dcs"][:, :], in_=T["cs"][:, :], func=Act.Copy,
                           scale=DT)
            act.activation(out=T["ksw"][:, :], in_=T["swp"][:, :], func=Act.Copy,
                           scale=K_COUP)
            act.activation(out=T["dsw"][:, :], in_=T["swp"][:, :], func=Act.Copy,
                           scale=DT).then_inc(a_s)

        @block.tensor
        def _(pe):
            pe.wait_ge(dma_c, 16)
            pe.matmul(warm[:, :], upT, inp[:, 0:1])
            pe.wait_ge(v2, 1)
            # cps = ones.T @ s0cols  (+)  upT.T @ lastc  ->  [S0 + carry]
            pe.matmul(cps[:, :], onesM, T["s0cols"][:, :], start=True,
                      stop=False)
            pe.wait_ge(v3, 1)
            pe.matmul(cps[:, :], upT, T["lastc"][:, :], start=False, stop=True
                      ).then_inc(p_s)

        @block.vector
        def _(v):
            q = Seq(v)
            t = lambda n: T[n][:, :]

            def TS(out, in0, s1, op0, s2=None, op1=None, reads=(), writes=(),
                   inc=None):
                def emit():
                    if op1 is not None:
                        return v.tensor_scalar(out=out, in0=in0, scalar1=s1,
                                               scalar2=s2, op0=op0, op1=op1)
                    return v.tensor_scalar(out=out, in0=in0, scalar1=s1,
                                           scalar2=s2, op0=op0)
                q.op(emit, reads, writes, inc)

            def STT(out, in0, sc, in1, op0, op1, reads=(), writes=(), inc=None):
                q.op(lambda: v.scalar_tensor_tensor(
                    out=out, in0=in0, scalar=sc, in1=in1, op0=op0, op1=op1),
                    reads, writes, inc)

            def TT(out, in0, in1, op, reads=(), writes=(), inc=None):
                q.op(lambda: v.tensor_tensor(out=out, in0=in0, in1=in1, op=op),
                     reads, writes, inc)

            v.wait_ge(dma_s, 16)
            # --- trig args: sarg=wrap(phase); carg2=wrap(phase+pi/2) ---
            TS(t("p2"), phase, PI / 2, Alu.add, writes=["p2"])
            TS(t("m1s"), phase, PI, Alu.is_gt, writes=["m1s"])
            TS(t("m2s"), phase, -PI, Alu.is_lt, writes=["m2s"])
            TS(t("m1c"), phase, PI / 2, Alu.is_gt, writes=["m1c"])
            TS(t("m2c"), phase, -1.5 * PI, Alu.is_lt, writes=["m2c"])
            STT(t("sargA"), t("m1s"), -2 * PI, phase, Alu.mult, Alu.add,
                reads=["m1s"], writes=["sargA"])
            STT(t("cargA"), t("m1c"), -2 * PI, t("p2"), Alu.mult, Alu.add,
                reads=["m1c", "p2"], writes=["cargA"])
            STT(R("targ"), t("m2s"), 2 * PI, t("sargA"), Alu.mult, Alu.add,
                reads=["m2s", "sargA"], writes=["targ"])
            STT(L("targ"), t("m2c"), 2 * PI, t("cargA"), Alu.mult, Alu.add,
                reads=["m2c", "cargA"], writes=["targ"], inc=v1)

            # --- c/s-independent precompute (overlaps ACT) ---
            v.wait_ge(dma_b, 16)
            TT(t("sqp"), xy_pk, xy_pk, Alu.mult, writes=["sqp"])
            TS(t("negd"), xdo_pk[:, 0:F], -1.0, Alu.mult, writes=["negd"])
            TS(t("n1"), xdo_pk[:, 0:F], EPS, Alu.add, writes=["n1"])
            TT(t("r2"), L("sqp"), R("sqp"), Alu.add, reads=["sqp"],
               writes=["r2"])
            TT(t("d1"), t("negd"), xdo_pk[:, 0:F], Alu.max, reads=["negd"],
               writes=["d1"])
            q.op(lambda: v.memset(t("zeros"), 0.0), writes=["zeros"])
            TT(t("asq"), t("r2"), t("r2"), Alu.mult, reads=["r2"],
               writes=["asq"])
            TS(t("d1e"), t("d1"), EPS, Alu.add, reads=["d1"], writes=["d1e"])
            TS(t("a"), t("asq"), -ALPHA, Alu.mult, ALPHA, Alu.add,
               reads=["asq"], writes=["a"])
            q.op(lambda: v.reciprocal(t("rd"), t("d1e")), reads=["d1e"],
                 writes=["rd"])
            TT(t("t3"), t("a"), x, Alu.mult, reads=["a"], writes=["t3"])
            TT(t("ratio"), t("n1"), t("rd"), Alu.mult, reads=["n1", "rd"],
               writes=["ratio"])
            TT(t("t4"), t("a"), y, Alu.mult, reads=["a"], writes=["t4"])
            TT(t("hr"), ha, t("ratio"), Alu.mult, reads=["ratio"], writes=["hr"])
            TS(t("zeta"), t("hr"), -1.0, Alu.mult, 1.0 + EPS, Alu.add,
               reads=["hr"], writes=["zeta"])
            q.op(lambda: v.reciprocal(t("rz"), t("zeta")), reads=["zeta"],
                 writes=["rz"])
            TT(t("bt"), wfr, t("rz"), Alu.mult, reads=["rz"], writes=["bt"])
            TT(t("t5"), t("bt"), y, Alu.mult, reads=["bt"], writes=["t5"])
            TT(t("t6"), t("bt"), x, Alu.mult, reads=["bt"], writes=["t6"])
            TT(t("vx"), t("t3"), t("t5"), Alu.subtract, reads=["t3", "t5"],
               writes=["vx"])
            TT(t("vy"), t("t6"), t("t4"), Alu.add, reads=["t6", "t4"],
               writes=["vy"])

            # --- e = v - k*xy: iteration-0 dot basis.  The k*P@S0 term is
            # dropped from iteration 0 (its effect on the final output is
            # ~1e-8, below fp32) and S0 is instead folded into the carry
            # matmul, so iteration 0 has NO PE dependency at all. ---
            STT(L("qp"), x, -K_COUP, t("vx"), Alu.mult, Alu.add,
                reads=["vx"], writes=["qp"])
            STT(R("qp"), y, -K_COUP, t("vy"), Alu.mult, Alu.add,
                reads=["vy"], writes=["qp"])
            # dot0/z/scan chain, with the S0 column-sum ops (needed only by
            # the PE matmul) interleaved as the RAW-distance fillers
            v.wait_ge(a_s, 1)
            TT(t("dot"), t("qp"), t("lo"), Alu.max, reads=["qp"],
               writes=["dot"])
            TT(t("P1"), t("cs"), xy_pk, Alu.mult, writes=["P1"])
            TT(t("dot"), t("dot"), t("hi"), Alu.min, reads=["dot"],
               writes=["dot"])
            TS(R("P2"), x, -1.0, Alu.mult, writes=["P2"])
            q.op(lambda: v.tensor_copy(L("P2"), y), writes=["P2"])
            v.wait_ge(a_s, 2)
            TT(t("A"), t("dcs"), t("dot"), Alu.mult, reads=["dot"],
               writes=["A"])
            q.op(lambda: v.tensor_reduce(T["s0cols"][:, 0:1], t("P1"), AxX,
                                         Alu.add),
                 reads=["P1"], writes=["s0cols"])
            TT(t("B"), t("dsw"), t("dot"), Alu.mult, reads=["dot"],
               writes=["B"])
            TT(t("uAB"), t("cs"), t("P2"), Alu.mult, reads=["P2"],
               writes=["uAB"])
            TT(t("zx"), L("A"), R("A"), Alu.add, reads=["A"], writes=["zx"])
            q.op(lambda: v.tensor_reduce(T["s0cols"][:, 1:2], t("uAB"), AxX,
                                         Alu.add),
                 reads=["uAB"], writes=["s0cols"], inc=v2)
            TT(t("zy"), R("B"), L("B"), Alu.subtract, reads=["B"],
               writes=["zy"])
            # per-partition z totals via reduce (not the scan tails) so the
            # PE carry matmul overlaps the scans below
            q.op(lambda: v.tensor_reduce(T["lastc"][:, 0:1], t("zx"), AxX,
                                         Alu.add),
                 reads=["zx"], writes=["lastc"])
            q.op(lambda: v.tensor_reduce(T["lastc"][:, 1:2], t("zy"), AxX,
                                         Alu.add),
                 reads=["zy"], writes=["lastc"], inc=v3)
            q.op(lambda: v.tensor_tensor_scan(
                out=L("incl"), data0=t("zx"), data1=t("zeros"), initial=0.0,
                op0=Alu.add, op1=Alu.add),
                reads=["zx", "zeros"], writes=["incl"])
            q.op(lambda: v.tensor_tensor_scan(
                out=R("incl"), data0=t("zy"), data1=t("zeros"), initial=0.0,
                op0=Alu.add, op1=Alu.add),
                reads=["zy", "zeros"], writes=["incl"])

            # --- D+S0 = excl prefix + S0 (single accumulated PE psum) ---
            v.wait_ge(p_s, 1)
            STT(L("Dp"), L("incl"), cps[:, 0:1], t("zx"), Alu.add,
                Alu.subtract, reads=["incl", "zx"], writes=["Dp"])
            STT(R("Dp"), R("incl"), cps[:, 1:2], t("zy"), Alu.add,
                Alu.subtract, reads=["incl", "zy"], writes=["Dp"])
            TT(t("A"), t("kcs"), t("Dp"), Alu.mult, reads=["Dp"], writes=["A"])
            TT(t("B"), t("ksw"), t("Dp"), Alu.mult, reads=["Dp"], writes=["B"])
            TT(L("f"), L("A"), R("A"), Alu.subtract, reads=["A"], writes=["f"])
            TT(R("f"), L("B"), R("B"), Alu.add, reads=["B"], writes=["f"])
            TT(t("f"), t("f"), t("qp"), Alu.add, reads=["f", "qp"],
               writes=["f"])
            TT(t("dot"), t("f"), t("lo"), Alu.max, reads=["f"], writes=["dot"])
            TT(t("dot"), t("dot"), t("hi"), Alu.min, reads=["dot"],
               writes=["dot"])
            # angles = amp * (y + DT*doty) + b
            STT(t("ynew"), R("dot"), DT, y, Alu.mult, Alu.add,
                reads=["dot"], writes=["ynew"])
            TT(t("anga"), amp, t("ynew"), Alu.mult, reads=["ynew"],
               writes=["anga"])
            TT(t("ang"), t("anga"), bofs, Alu.add, reads=["anga"],
               writes=["ang"], inc=v_done)

    ctx.close()
    _strip_init_barrier(nc)
    return nc


def _strip_init_barrier(nc):
    """Remove the Bass-init all-engine rendezvous (4 Drains + EVSEM butterfly,
    ~3us) from the entry block.  Every ordering this kernel needs flows through
    its explicit semaphores: the Pool const-memsets precede the input DMA in
    Pool program order and all other engines gate on dma_s, so the rendezvous
    is redundant.  The Block-exit barrier is left untouched (removing it was
    observed to race)."""
    bb = nc.main_func.blocks[0]
    keep = [ins for ins in bb.instructions
            if not (type(ins).__name__ == "InstDrain"
                    or (type(ins).__name__ == "InstEventSemaphore"
                        and "barrier" in ins.name))]
    if len(keep) != len(bb.instructions):
        del bb.instructions[:]
        for ins in keep:
            bb.instructions.append(ins)


def _get_nc():
    if "nc" not in _CACHE:
        _CACHE["nc"] = _build()
    return _CACHE["nc"]


def pack_inputs(phase, amplitudes, w, ha, b, xy, xy_dot_old):
    f = np.float32
    xy = np.asarray(xy, f)
    xdo = np.asarray(xy_dot_old, f)
    planes = [
        np.asarray(phase, f).reshape(P, F),
        np.asarray(amplitudes, f).reshape(P, F),
        np.asarray(w, f).reshape(P, F),
        np.asarray(ha, f).reshape(P, F),
        np.asarray(b, f).reshape(P, F),
        np.ascontiguousarray(xy[:, 0]).reshape(P, F),
        np.ascontiguousarray(xy[:, 1]).reshape(P, F),
        np.ascontiguousarray(xdo[:, 0]).reshape(P, F),
        np.ascontiguousarray(xdo[:, 1]).reshape(P, F),
        np.triu(np.ones((P, P), f), k=1),
        np.ones((P, P), f),
    ]
    return {"inp": np.ascontiguousarray(np.concatenate(planes, axis=1))}


def kernel(phase, amplitudes, w, ha, b, xy, xy_dot_old, adj_mask):
    from concourse.bass_utils import run_bass_kernel_spmd

    nc = _get_nc()
    in_map = pack_inputs(phase, amplitudes, w, ha, b, xy, xy_dot_old)
    n_cores = 8
    res = run_bass_kernel_spmd(nc, [in_map] * n_cores, core_ids=list(range(n_cores)))
    return np.asarray(res.results[0]["angles"], dtype=np.float32).reshape(N)


# revision 32
# speedup vs baseline: 1.2137x; 1.0217x over previous
"""Trainium2 Bass kernel for nn_BodyAgnosticNACPG (N=4096 coupled oscillators,
fully-connected Gauss-Seidel sweep).

Math: R[i,j] = rot(phase_i - phase_j) = rot(phase_i) @ rot(-phase_j), and the
adjacency is complete-minus-self, so the coupling sum for oscillator i is
    coup_i = (COUP/deg) * rot(phase_i) @ (S_i - u_i),   u_j = rot(-phase_j) @ xy_j
with S_i = sum_j u_j^(current).  Updating i changes S by DT*rot(-phase_i)@dot_i,
so with z_j = DT*G_j dot_j and D_i = sum_{j<i} z_j (exclusive prefix):
    dot_i = clip(q_i + k*P_i @ D_i, lo_i, hi_i)
    q_i   = K_i x_i - k*x_i + k*P_i @ S0      (all precomputable in parallel)
The k = COUP/4095 ~ 2e-5 coupling makes the fixed point contract at ~8e-4 per
sweep, so 2 evaluations (one prefix-sum round) reach the fp32 noise floor.

On-device layout: [128 partition x 32 free], element i -> [i//32, i%32]; the
x/y components of most intermediates are packed side by side in [128, 64]
tiles so one Vector op handles both.  The exclusive prefix sum is a
per-partition tensor_tensor_scan plus one cross-partition carry matmul
(strict-upper-triangular ones, rhs [128,2] = both components); the S0
partition-reduce-and-broadcast is one matmul with an all-ones matrix.

Written in raw Bass (BSP Block + explicit semaphores) because this
toolchain's walrus rejects TileContext's tail drain (its multi-sem-wait CTRL
instruction exceeds the 1-wait ISA slot).  Hardware quirks measured on this
silicon and reflected here:
  * A DVE instruction reading a tensor written by the immediately preceding
    DVE instruction sees stale data (no interlock at distance 1; distance 2
    measured safe).  The Seq helper enforces read-after-write distance >= 3,
    inserting memset spacers when the natural interleave isn't enough.
  * tensor_max (the method) and stt accum_out are broken; tensor_tensor
    (op=max/min) and tensor_reduce are used instead.
  * GpSimd affine_select deadlocks against concurrent DVE work, so the
    triangular/ones matrices ship with the input DMA (second, non-blocking
    transfer) instead of being built on-device.
Engine split: Pool(gpsimd) runs the DMAs; ACT prewarms the Sin table during
the DMA, computes both sines in ONE packed activation (cos(p) = sin(p+pi/2)
folded into the range reduction), and produces the scaled trig copies and
clip bounds off the critical path; PE does one warmup + 2 batched matmuls;
DVE runs the ~70-op main chain.  Each instruction carries at most one
semaphore wait.

The whole problem is ~200KB of data and O(n) flops, so each of the 8 cores
redundantly computes the full answer (no collectives); core 0's output is
returned.  adj_mask is all-ones by construction (deg = n-1 hardcoded) and
never touches the device.
"""

import numpy as np

N = 4096
P = 128
F = 32  # free dim: N = P * F, element i -> [i // F, i % F]
F2 = 2 * F
NPLANES = 9
WIDE = NPLANES * F + 2 * P  # 9 input planes + strict-upper-tri ones + all-ones

ALPHA = 0.45
DT = 0.01
COUP = 0.08
DIFF = 10.0
EPS = 1e-9
K_COUP = float(np.float32(COUP) / np.float32(N - 1))
PI = float(np.pi)

MIN_RAW_DIST = 2  # measured: dist-1 RAW is broken, dist-2 safe

_CACHE = {}


def _build():
    from contextlib import ExitStack
    import concourse.bass as bass
    import concourse.mybir as mybir

    f32 = mybir.dt.float32
    Act = mybir.ActivationFunctionType
    Alu = mybir.AluOpType
    AxX = mybir.AxisListType.X

    nc = bass.Bass("TRN2", debug=False, target_bir_lowering=False)

    d_inp = nc.dram_tensor("inp", [P, WIDE], f32, kind="ExternalInput")
    d_out = nc.dram_tensor("angles", [P, F], f32, kind="ExternalOutput")

    ctx = ExitStack()
    sem = lambda name: ctx.enter_context(nc.semaphore(name))
    sb = lambda name, w=F: ctx.enter_context(nc.sbuf_tensor(name, [P, w], f32))

    dma_s = sem("dma_s")
    dma_b = sem("dma_b")    # planes 1-8
    dma_c = sem("dma_c")    # matrices
    v1 = sem("v1")          # DVE: trig args ready
    a_s = sem("a_s")        # ACT: 1 = sines, 2 = all scaled copies/bounds
    v2 = sem("v2")          # DVE: s0 columns ready
    p_s = sem("p_s")        # PE: 1 = s0 matmul, 2 = carry matmul
    v3 = sem("v3")          # DVE: incl scans ready
    v_done = sem("v_done")  # DVE: output ready

    inp = ctx.enter_context(nc.sbuf_tensor("inpt", [P, WIDE], f32))
    # [128,64] packed tiles (x-half | y-half unless noted)
    packs = """targ cs swp kcs dcs ksw dsw sqp P1 P2 uAB lo hi qp A B f dot
        Dp incl""".split()
    T = {n: sb(n, F2) for n in packs}
    for n in """sargA cargA p2 m1s m2s m1c m2c
        r2 asq a n1 negd d1 d1e rd ratio hr zeta rz bt
        t3 t4 t5 t6 vx vy e1 e2 zx zy
        ynew anga ang zeros spacer""".split():
        T[n] = sb(n)
    T["s0cols"] = sb("s0cols", 2)
    T["lastc"] = sb("lastc", 2)

    psum = lambda name, w: ctx.enter_context(nc.psum_tensor(name, [P, w], f32))
    warm = psum("warm", 1)
    cps = psum("cps", 2)    # [S0x + carry_x | S0y + carry_y] per partition

    def plane(i):
        return inp[:, i * F:(i + 1) * F]

    phase = plane(0); amp = plane(1); wfr = plane(2); ha = plane(3)
    bofs = plane(4); x = plane(5); y = plane(6)
    xy_pk = inp[:, 5 * F:7 * F]    # [x|y]
    xdo_pk = inp[:, 7 * F:9 * F]   # [xdx|xdy]
    upT = inp[:, NPLANES * F:NPLANES * F + P]           # U[k,m]=1 iff k<m
    onesM = inp[:, NPLANES * F + P:NPLANES * F + 2 * P]  # all ones

    def L(n):   # left (x) half of a pack
        return T[n][:, 0:F]

    def R(n):   # right (y) half of a pack
        return T[n][:, F:F2]

    class Seq:
        """Emit DVE ops enforcing intra-engine RAW distance >= MIN_RAW_DIST."""

        def __init__(self, v):
            self.v = v
            self.pos = 0
            self.last_w = {}
            self.n_spacers = 0

        def op(self, fn, reads=(), writes=(), inc=None):
            while any(self.pos - self.last_w.get(r, -10) < MIN_RAW_DIST
                      for r in reads):
                self.v.memset(T["spacer"][:, 0:F], 0.0)
                self.pos += 1
                self.n_spacers += 1
            inst = fn()
            if inc is not None:
                inst.then_inc(inc)
            for w in writes:
                self.last_w[w] = self.pos
            self.pos += 1

    with nc.Block(no_gpsimd_drain=True) as block:

        @block.gpsimd
        def _(g):
            NF = NPLANES * F
            # phase plane first: unblocks the DVE wrap + ACT Sin early
            g.dma_start(out=inp[:, 0:F], in_=d_inp[:, 0:F]).then_inc(dma_s, 16)
            g.dma_start(out=inp[:, F:NF], in_=d_inp[:, F:NF]).then_inc(dma_b, 16)
            g.dma_start(out=inp[:, NF:WIDE], in_=d_inp[:, NF:WIDE]
                        ).then_inc(dma_c, 16)
            g.wait_ge(v_done, 1)
            g.dma_start(out=d_out[:, :], in_=T["ang"][:, :]).then_inc(dma_s, 32)
            g.wait_ge(dma_s, 48)

        @block.scalar
        def _(act):
            # dummy Sin: pulls the ACT table while the input DMA runs
            act.activation(out=T["lo"][:, 0:1], in_=T["lo"][:, 0:1],
                           func=Act.Sin)
            act.wait_ge(dma_b, 16)
            # clip bounds (Copy with +-DIFF bias), off the DVE critical path
            act.activation(out=T["lo"][:, :], in_=xdo_pk, func=Act.Copy,
                           bias=-DIFF)
            act.activation(out=T["hi"][:, :], in_=xdo_pk, func=Act.Copy,
                           bias=DIFF)
            act.wait_ge(v1, 1)
            # targ = [carg+pi/2 | sarg]  ->  cs = [cos(phase) | sin(phase)]
            act.activation(out=T["cs"][:, :], in_=T["targ"][:, :], func=Act.Sin
                           ).then_inc(a_s)
            # swapped and scaled copies: swp=[s|c], kcs=k*[c|s], dcs=DT*[c|s],
            # ksw=k*[s|c], dsw=DT*[s|c]
            act.activation(out=L("swp"), in_=R("cs"), func=Act.Copy)
            act.activation(out=R("swp"), in_=L("cs"), func=Act.Copy)
            act.activation(out=T["kcs"][:, :], in_=T["cs"][:, :], func=Act.Copy,
                           scale=K_COUP)
            act.activation(out=T["dcs"][:, :], in_=T["cs"][:, :], func=Act.Copy,
                           scale=DT)
            act.activation(out=T["ksw"][:, :], in_=T["swp"][:, :], func=Act.Copy,
                           scale=K_COUP)
            act.activation(out=T["dsw"][:, :], in_=T["swp"][:, :], func=Act.Copy,
                           scale=DT).then_inc(a_s)

        @block.tensor
        def _(pe):
            pe.wait_ge(dma_c, 16)
            pe.matmul(warm[:, :], upT, inp[:, 0:1])
            pe.wait_ge(v2, 1)
            # cps = ones.T @ s0cols  (+)  upT.T @ lastc  ->  [S0 + carry]
            pe.matmul(cps[:, :], onesM, T["s0cols"][:, :], start=True,
                      stop=False)
            pe.wait_ge(v3, 1)
            pe.matmul(cps[:, :], upT, T["lastc"][:, :], start=False, stop=True
                      ).then_inc(p_s)

        @block.vector
        def _(v):
            q = Seq(v)
            t = lambda n: T[n][:, :]

            def TS(out, in0, s1, op0, s2=None, op1=None, reads=(), writes=(),
                   inc=None):
                def emit():
                    if op1 is not None:
                        return v.tensor_scalar(out=out, in0=in0, scalar1=s1,
                                               scalar2=s2, op0=op0, op1=op1)
                    return v.tensor_scalar(out=out, in0=in0, scalar1=s1,
                                           scalar2=s2, op0=op0)
                q.op(emit, reads, writes, inc)

            def STT(out, in0, sc, in1, op0, op1, reads=(), writes=(), inc=None):
                q.op(lambda: v.scalar_tensor_tensor(
                    out=out, in0=in0, scalar=sc, in1=in1, op0=op0, op1=op1),
                    reads, writes, inc)

            def TT(out, in0, in1, op, reads=(), writes=(), inc=None):
                q.op(lambda: v.tensor_tensor(out=out, in0=in0, in1=in1, op=op),
                     reads, writes, inc)

            v.wait_ge(dma_s, 16)
            # --- trig args: sarg=wrap(phase); carg2=wrap(phase+pi/2) ---
            TS(t("p2"), phase, PI / 2, Alu.add, writes=["p2"])
            TS(t("m1s"), phase, PI, Alu.is_gt, writes=["m1s"])
            TS(t("m2s"), phase, -PI, Alu.is_lt, writes=["m2s"])
            TS(t("m1c"), phase, PI / 2, Alu.is_gt, writes=["m1c"])
            TS(t("m2c"), phase, -1.5 * PI, Alu.is_lt, writes=["m2c"])
            STT(t("sargA"), t("m1s"), -2 * PI, phase, Alu.mult, Alu.add,
                reads=["m1s"], writes=["sargA"])
            STT(t("cargA"), t("m1c"), -2 * PI, t("p2"), Alu.mult, Alu.add,
                reads=["m1c", "p2"], writes=["cargA"])
            STT(R("targ"), t("m2s"), 2 * PI, t("sargA"), Alu.mult, Alu.add,
                reads=["m2s", "sargA"], writes=["targ"])
            STT(L("targ"), t("m2c"), 2 * PI, t("cargA"), Alu.mult, Alu.add,
                reads=["m2c", "cargA"], writes=["targ"], inc=v1)

            # --- c/s-independent precompute (overlaps ACT) ---
            v.wait_ge(dma_b, 16)
            TT(t("sqp"), xy_pk, xy_pk, Alu.mult, writes=["sqp"])
            TS(t("negd"), xdo_pk[:, 0:F], -1.0, Alu.mult, writes=["negd"])
            TS(t("n1"), xdo_pk[:, 0:F], EPS, Alu.add, writes=["n1"])
            TT(t("r2"), L("sqp"), R("sqp"), Alu.add, reads=["sqp"],
               writes=["r2"])
            TT(t("d1"), t("negd"), xdo_pk[:, 0:F], Alu.max, reads=["negd"],
               writes=["d1"])
            q.op(lambda: v.memset(t("zeros"), 0.0), writes=["zeros"])
            TT(t("asq"), t("r2"), t("r2"), Alu.mult, reads=["r2"],
               writes=["asq"])
            TS(t("d1e"), t("d1"), EPS, Alu.add, reads=["d1"], writes=["d1e"])
            TS(t("a"), t("asq"), -ALPHA, Alu.mult, ALPHA, Alu.add,
               reads=["asq"], writes=["a"])
            q.op(lambda: v.reciprocal(t("rd"), t("d1e")), reads=["d1e"],
                 writes=["rd"])
            TT(t("t3"), t("a"), x, Alu.mult, reads=["a"], writes=["t3"])
            TT(t("ratio"), t("n1"), t("rd"), Alu.mult, reads=["n1", "rd"],
               writes=["ratio"])
            TT(t("t4"), t("a"), y, Alu.mult, reads=["a"], writes=["t4"])
            TT(t("hr"), ha, t("ratio"), Alu.mult, reads=["ratio"], writes=["hr"])
            TS(t("zeta"), t("hr"), -1.0, Alu.mult, 1.0 + EPS, Alu.add,
               reads=["hr"], writes=["zeta"])
            q.op(lambda: v.reciprocal(t("rz"), t("zeta")), reads=["zeta"],
                 writes=["rz"])
            TT(t("bt"), wfr, t("rz"), Alu.mult, reads=["rz"], writes=["bt"])
            TT(t("t5"), t("bt"), y, Alu.mult, reads=["bt"], writes=["t5"])
            TT(t("t6"), t("bt"), x, Alu.mult, reads=["bt"], writes=["t6"])
            TT(t("vx"), t("t3"), t("t5"), Alu.subtract, reads=["t3", "t5"],
               writes=["vx"])
            TT(t("vy"), t("t6"), t("t4"), Alu.add, reads=["t6", "t4"],
               writes=["vy"])

            # --- e = v - k*xy: iteration-0 dot basis.  The k*P@S0 term is
            # dropped from iteration 0 (its effect on the final output is
            # ~1e-8, below fp32) and S0 is instead folded into the carry
            # matmul, so iteration 0 has NO PE dependency at all. ---
            STT(L("qp"), x, -K_COUP, t("vx"), Alu.mult, Alu.add,
                reads=["vx"], writes=["qp"])
            STT(R("qp"), y, -K_COUP, t("vy"), Alu.mult, Alu.add,
                reads=["vy"], writes=["qp"])
            # dot0/z/scan chain, with the S0 column-sum ops (needed only by
            # the PE matmul) interleaved as the RAW-distance fillers
            v.wait_ge(a_s, 1)
            TT(t("dot"), t("qp"), t("lo"), Alu.max, reads=["qp"],
               writes=["dot"])
            TT(t("P1"), t("cs"), xy_pk, Alu.mult, writes=["P1"])
            TT(t("dot"), t("dot"), t("hi"), Alu.min, reads=["dot"],
               writes=["dot"])
            TS(R("P2"), x, -1.0, Alu.mult, writes=["P2"])
            q.op(lambda: v.tensor_copy(L("P2"), y), writes=["P2"])
            v.wait_ge(a_s, 2)
            TT(t("A"), t("dcs"), t("dot"), Alu.mult, reads=["dot"],
               writes=["A"])
            q.op(lambda: v.tensor_reduce(T["s0cols"][:, 0:1], t("P1"), AxX,
                                         Alu.add),
                 reads=["P1"], writes=["s0cols"])
            TT(t("B"), t("dsw"), t("dot"), Alu.mult, reads=["dot"],
               writes=["B"])
            TT(t("uAB"), t("cs"), t("P2"), Alu.mult, reads=["P2"],
               writes=["uAB"])
            TT(t("zx"), L("A"), R("A"), Alu.add, reads=["A"], writes=["zx"])
            q.op(lambda: v.tensor_reduce(T["s0cols"][:, 1:2], t("uAB"), AxX,
                                         Alu.add),
                 reads=["uAB"], writes=["s0cols"], inc=v2)
            TT(t("zy"), R("B"), L("B"), Alu.subtract, reads=["B"],
               writes=["zy"])
            # per-partition z totals via reduce (not the scan tails) so the
            # PE carry matmul overlaps the scans below
            q.op(lambda: v.tensor_reduce(T["lastc"][:, 0:1], t("zx"), AxX,
                                         Alu.add),
                 reads=["zx"], writes=["lastc"])
            q.op(lambda: v.tensor_reduce(T["lastc"][:, 1:2], t("zy"), AxX,
                                         Alu.add),
                 reads=["zy"], writes=["lastc"], inc=v3)
            q.op(lambda: v.tensor_tensor_scan(
                out=L("incl"), data0=t("zx"), data1=t("zeros"), initial=0.0,
                op0=Alu.add, op1=Alu.add),
                reads=["zx", "zeros"], writes=["incl"])
            q.op(lambda: v.tensor_tensor_scan(
                out=R("incl"), data0=t("zy"), data1=t("zeros"), initial=0.0,
                op0=Alu.add, op1=Alu.add),
                reads=["zy", "zeros"], writes=["incl"])

            # --- D+S0 = excl prefix + S0 (single accumulated PE psum) ---
            v.wait_ge(p_s, 1)
            STT(L("Dp"), L("incl"), cps[:, 0:1], t("zx"), Alu.add,
                Alu.subtract, reads=["incl", "zx"], writes=["Dp"])
            STT(R("Dp"), R("incl"), cps[:, 1:2], t("zy"), Alu.add,
                Alu.subtract, reads=["incl", "zy"], writes=["Dp"])
            TT(t("A"), t("kcs"), t("Dp"), Alu.mult, reads=["Dp"], writes=["A"])
            TT(t("B"), t("ksw"), t("Dp"), Alu.mult, reads=["Dp"], writes=["B"])
            TT(L("f"), L("A"), R("A"), Alu.subtract, reads=["A"], writes=["f"])
            TT(R("f"), L("B"), R("B"), Alu.add, reads=["B"], writes=["f"])
            TT(t("f"), t("f"), t("qp"), Alu.add, reads=["f", "qp"],
               writes=["f"])
            TT(t("dot"), t("f"), t("lo"), Alu.max, reads=["f"], writes=["dot"])
            TT(t("dot"), t("dot"), t("hi"), Alu.min, reads=["dot"],
               writes=["dot"])
            # angles = amp * (y + DT*doty) + b
            STT(t("ynew"), R("dot"), DT, y, Alu.mult, Alu.add,
                reads=["dot"], writes=["ynew"])
            TT(t("anga"), amp, t("ynew"), Alu.mult, reads=["ynew"],
               writes=["anga"])
            TT(t("ang"), t("anga"), bofs, Alu.add, reads=["anga"],
               writes=["ang"], inc=v_done)

    ctx.close()
    _strip_init_barrier(nc)
    return nc


def _strip_init_barrier(nc):
    """Remove the Bass-init all-engine rendezvous (4 Drains + EVSEM butterfly,
    ~3us) from the entry block.  Every ordering this kernel needs flows through
    its explicit semaphores: the Pool const-memsets precede the input DMA in
    Pool program order and all other engines gate on dma_s, so the rendezvous
    is redundant.  The Block-exit barrier is left untouched (removing it was
    observed to race)."""
    bb = nc.main_func.blocks[0]
    keep = [ins for ins in bb.instructions
            if not (type(ins).__name__ == "InstDrain"
                    or (type(ins).__name__ == "InstEventSemaphore"
                        and "barrier" in ins.name))]
    if len(keep) != len(bb.instructions):
        del bb.instructions[:]
        for ins in keep:
            bb.instructions.append(ins)


def _get_nc():
    if "nc" not in _CACHE:
        _CACHE["nc"] = _build()
    return _CACHE["nc"]


def pack_inputs(phase, amplitudes, w, ha, b, xy, xy_dot_old):
    f = np.float32
    xy = np.asarray(xy, f)
    xdo = np.asarray(xy_dot_old, f)
    planes = [
        np.asarray(phase, f).reshape(P, F),
        np.asarray(amplitudes, f).reshape(P, F),
        np.asarray(w, f).reshape(P, F),
        np.asarray(ha, f).reshape(P, F),
        np.asarray(b, f).reshape(P, F),
        np.ascontiguousarray(xy[:, 0]).reshape(P, F),
        np.ascontiguousarray(xy[:, 1]).reshape(P, F),
        np.ascontiguousarray(xdo[:, 0]).reshape(P, F),
        np.ascontiguousarray(xdo[:, 1]).reshape(P, F),
        np.triu(np.ones((P, P), f), k=1),
        np.ones((P, P), f),
    ]
    return {"inp": np.ascontiguousarray(np.concatenate(planes, axis=1))}


def kernel(phase, amplitudes, w, ha, b, xy, xy_dot_old, adj_mask):
    from concourse.bass_utils import run_bass_kernel_spmd

    nc = _get_nc()
    in_map = pack_inputs(phase, amplitudes, w, ha, b, xy, xy_dot_old)
    n_cores = 8
    res = run_bass_kernel_spmd(nc, [in_map] * n_cores, core_ids=list(range(n_cores)))
    return np.asarray(res.results[0]["angles"], dtype=np.float32).reshape(N)


# revision 33
# speedup vs baseline: 1.2161x; 1.0020x over previous
"""Trainium2 Bass kernel for nn_BodyAgnosticNACPG (N=4096 coupled oscillators,
fully-connected Gauss-Seidel sweep).

Math: R[i,j] = rot(phase_i - phase_j) = rot(phase_i) @ rot(-phase_j), and the
adjacency is complete-minus-self, so the coupling sum for oscillator i is
    coup_i = (COUP/deg) * rot(phase_i) @ (S_i - u_i),   u_j = rot(-phase_j) @ xy_j
with S_i = sum_j u_j^(current).  Updating i changes S by DT*rot(-phase_i)@dot_i,
so with z_j = DT*G_j dot_j and D_i = sum_{j<i} z_j (exclusive prefix):
    dot_i = clip(q_i + k*P_i @ D_i, lo_i, hi_i)
    q_i   = K_i x_i - k*x_i + k*P_i @ S0      (all precomputable in parallel)
The k = COUP/4095 ~ 2e-5 coupling makes the fixed point contract at ~8e-4 per
sweep, so 2 evaluations (one prefix-sum round) reach the fp32 noise floor.

On-device layout: [128 partition x 32 free], element i -> [i//32, i%32]; the
x/y components of most intermediates are packed side by side in [128, 64]
tiles so one Vector op handles both.  The exclusive prefix sum is a
per-partition tensor_tensor_scan plus one cross-partition carry matmul
(strict-upper-triangular ones, rhs [128,2] = both components); the S0
partition-reduce-and-broadcast is one matmul with an all-ones matrix.

Written in raw Bass (BSP Block + explicit semaphores) because this
toolchain's walrus rejects TileContext's tail drain (its multi-sem-wait CTRL
instruction exceeds the 1-wait ISA slot).  Hardware quirks measured on this
silicon and reflected here:
  * A DVE instruction reading a tensor written by the immediately preceding
    DVE instruction sees stale data (no interlock at distance 1; distance 2
    measured safe).  The Seq helper enforces read-after-write distance >= 3,
    inserting memset spacers when the natural interleave isn't enough.
  * tensor_max (the method) and stt accum_out are broken; tensor_tensor
    (op=max/min) and tensor_reduce are used instead.
  * GpSimd affine_select deadlocks against concurrent DVE work, so the
    triangular/ones matrices ship with the input DMA (second, non-blocking
    transfer) instead of being built on-device.
Engine split: Pool(gpsimd) runs the DMAs; ACT prewarms the Sin table during
the DMA, computes both sines in ONE packed activation (cos(p) = sin(p+pi/2)
folded into the range reduction), and produces the scaled trig copies and
clip bounds off the critical path; PE does one warmup + 2 batched matmuls;
DVE runs the ~70-op main chain.  Each instruction carries at most one
semaphore wait.

The whole problem is ~200KB of data and O(n) flops, so each of the 8 cores
redundantly computes the full answer (no collectives); core 0's output is
returned.  adj_mask is all-ones by construction (deg = n-1 hardcoded) and
never touches the device.
"""

import numpy as np

N = 4096
P = 128
F = 32  # free dim: N = P * F, element i -> [i // F, i % F]
F2 = 2 * F
NPLANES = 9
WIDE = NPLANES * F + 2 * P  # 9 input planes + strict-upper-tri ones + all-ones

ALPHA = 0.45
DT = 0.01
COUP = 0.08
DIFF = 10.0
EPS = 1e-9
K_COUP = float(np.float32(COUP) / np.float32(N - 1))
PI = float(np.pi)

MIN_RAW_DIST = 2  # measured: dist-1 RAW is broken, dist-2 safe

_CACHE = {}


def _build():
    from contextlib import ExitStack
    import concourse.bass as bass
    import concourse.mybir as mybir

    f32 = mybir.dt.float32
    Act = mybir.ActivationFunctionType
    Alu = mybir.AluOpType
    AxX = mybir.AxisListType.X

    nc = bass.Bass("TRN2", debug=False, target_bir_lowering=False)

    d_inp = nc.dram_tensor("inp", [P, WIDE], f32, kind="ExternalInput")
    d_out = nc.dram_tensor("angles", [P, F], f32, kind="ExternalOutput")

    ctx = ExitStack()
    sem = lambda name: ctx.enter_context(nc.semaphore(name))
    sb = lambda name, w=F: ctx.enter_context(nc.sbuf_tensor(name, [P, w], f32))

    dma_s = sem("dma_s")
    dma_b = sem("dma_b")    # xy + xdo planes (5-8)
    dma_w = sem("dma_w")    # amp/w/ha/b planes (1-4)
    dma_c = sem("dma_c")    # matrices
    v1 = sem("v1")          # DVE: trig args ready
    a_s = sem("a_s")        # ACT: 1 = sines, 2 = all scaled copies/bounds
    v2 = sem("v2")          # DVE: s0 columns ready
    p_s = sem("p_s")        # PE: 1 = s0 matmul, 2 = carry matmul
    v3 = sem("v3")          # DVE: incl scans ready
    v_done = sem("v_done")  # DVE: output ready

    inp = ctx.enter_context(nc.sbuf_tensor("inpt", [P, WIDE], f32))
    # [128,64] packed tiles (x-half | y-half unless noted)
    packs = """targ cs swp kcs dcs ksw dsw sqp P1 P2 uAB lo hi qp A B f dot
        Dp incl""".split()
    T = {n: sb(n, F2) for n in packs}
    for n in """sargA cargA p2 m1s m2s m1c m2c
        r2 asq a n1 negd d1 d1e rd ratio hr zeta rz bt
        t3 t4 t5 t6 vx vy e1 e2 zx zy
        ynew anga ang zeros spacer""".split():
        T[n] = sb(n)
    T["s0cols"] = sb("s0cols", 2)
    T["lastc"] = sb("lastc", 2)

    psum = lambda name, w: ctx.enter_context(nc.psum_tensor(name, [P, w], f32))
    warm = psum("warm", 1)
    cps = psum("cps", 2)    # [S0x + carry_x | S0y + carry_y] per partition

    def plane(i):
        return inp[:, i * F:(i + 1) * F]

    phase = plane(0); amp = plane(1); wfr = plane(2); ha = plane(3)
    bofs = plane(4); x = plane(5); y = plane(6)
    xy_pk = inp[:, 5 * F:7 * F]    # [x|y]
    xdo_pk = inp[:, 7 * F:9 * F]   # [xdx|xdy]
    upT = inp[:, NPLANES * F:NPLANES * F + P]           # U[k,m]=1 iff k<m
    onesM = inp[:, NPLANES * F + P:NPLANES * F + 2 * P]  # all ones

    def L(n):   # left (x) half of a pack
        return T[n][:, 0:F]

    def R(n):   # right (y) half of a pack
        return T[n][:, F:F2]

    class Seq:
        """Emit DVE ops enforcing intra-engine RAW distance >= MIN_RAW_DIST."""

        def __init__(self, v):
            self.v = v
            self.pos = 0
            self.last_w = {}
            self.n_spacers = 0

        def op(self, fn, reads=(), writes=(), inc=None):
            while any(self.pos - self.last_w.get(r, -10) < MIN_RAW_DIST
                      for r in reads):
                self.v.memset(T["spacer"][:, 0:F], 0.0)
                self.pos += 1
                self.n_spacers += 1
            inst = fn()
            if inc is not None:
                inst.then_inc(inc)
            for w in writes:
                self.last_w[w] = self.pos
            self.pos += 1

    with nc.Block(no_gpsimd_drain=True) as block:

        @block.gpsimd
        def _(g):
            NF = NPLANES * F
            # phase plane first: unblocks the DVE wrap + ACT Sin early
            g.dma_start(out=inp[:, 0:F], in_=d_inp[:, 0:F]).then_inc(dma_s, 16)
            g.dma_start(out=inp[:, 5 * F:9 * F], in_=d_inp[:, 5 * F:9 * F]
                        ).then_inc(dma_b, 16)
            g.dma_start(out=inp[:, F:5 * F], in_=d_inp[:, F:5 * F]
                        ).then_inc(dma_w, 16)
            g.dma_start(out=inp[:, NF:WIDE], in_=d_inp[:, NF:WIDE]
                        ).then_inc(dma_c, 16)
            g.wait_ge(v_done, 1)
            g.dma_start(out=d_out[:, :], in_=T["ang"][:, :]).then_inc(dma_s, 32)
            g.wait_ge(dma_s, 48)

        @block.scalar
        def _(act):
            # dummy Sin: pulls the ACT table while the input DMA runs
            act.activation(out=T["lo"][:, 0:1], in_=T["lo"][:, 0:1],
                           func=Act.Sin)
            act.wait_ge(dma_b, 16)
            # clip bounds (Copy with +-DIFF bias), off the DVE critical path
            act.activation(out=T["lo"][:, :], in_=xdo_pk, func=Act.Copy,
                           bias=-DIFF)
            act.activation(out=T["hi"][:, :], in_=xdo_pk, func=Act.Copy,
                           bias=DIFF)
            act.wait_ge(v1, 1)
            # targ = [carg+pi/2 | sarg]  ->  cs = [cos(phase) | sin(phase)]
            act.activation(out=T["cs"][:, :], in_=T["targ"][:, :], func=Act.Sin
                           ).then_inc(a_s)
            # swapped and scaled copies: swp=[s|c], kcs=k*[c|s], dcs=DT*[c|s],
            # ksw=k*[s|c], dsw=DT*[s|c]
            act.activation(out=L("swp"), in_=R("cs"), func=Act.Copy)
            act.activation(out=R("swp"), in_=L("cs"), func=Act.Copy)
            act.activation(out=T["kcs"][:, :], in_=T["cs"][:, :], func=Act.Copy,
                           scale=K_COUP)
            act.activation(out=T["dcs"][:, :], in_=T["cs"][:, :], func=Act.Copy,
                           scale=DT)
            act.activation(out=T["ksw"][:, :], in_=T["swp"][:, :], func=Act.Copy,
                           scale=K_COUP)
            act.activation(out=T["dsw"][:, :], in_=T["swp"][:, :], func=Act.Copy,
                           scale=DT).then_inc(a_s)

        @block.tensor
        def _(pe):
            pe.wait_ge(dma_c, 16)
            pe.matmul(warm[:, :], upT, inp[:, 0:1])
            pe.wait_ge(v2, 1)
            # cps = ones.T @ s0cols  (+)  upT.T @ lastc  ->  [S0 + carry]
            pe.matmul(cps[:, :], onesM, T["s0cols"][:, :], start=True,
                      stop=False)
            pe.wait_ge(v3, 1)
            pe.matmul(cps[:, :], upT, T["lastc"][:, :], start=False, stop=True
                      ).then_inc(p_s)

        @block.vector
        def _(v):
            q = Seq(v)
            t = lambda n: T[n][:, :]

            def TS(out, in0, s1, op0, s2=None, op1=None, reads=(), writes=(),
                   inc=None):
                def emit():
                    if op1 is not None:
                        return v.tensor_scalar(out=out, in0=in0, scalar1=s1,
                                               scalar2=s2, op0=op0, op1=op1)
                    return v.tensor_scalar(out=out, in0=in0, scalar1=s1,
                                           scalar2=s2, op0=op0)
                q.op(emit, reads, writes, inc)

            def STT(out, in0, sc, in1, op0, op1, reads=(), writes=(), inc=None):
                q.op(lambda: v.scalar_tensor_tensor(
                    out=out, in0=in0, scalar=sc, in1=in1, op0=op0, op1=op1),
                    reads, writes, inc)

            def TT(out, in0, in1, op, reads=(), writes=(), inc=None):
                q.op(lambda: v.tensor_tensor(out=out, in0=in0, in1=in1, op=op),
                     reads, writes, inc)

            v.wait_ge(dma_s, 16)
            # --- trig args: sarg=wrap(phase); carg2=wrap(phase+pi/2) ---
            TS(t("p2"), phase, PI / 2, Alu.add, writes=["p2"])
            TS(t("m1s"), phase, PI, Alu.is_gt, writes=["m1s"])
            TS(t("m2s"), phase, -PI, Alu.is_lt, writes=["m2s"])
            TS(t("m1c"), phase, PI / 2, Alu.is_gt, writes=["m1c"])
            TS(t("m2c"), phase, -1.5 * PI, Alu.is_lt, writes=["m2c"])
            STT(t("sargA"), t("m1s"), -2 * PI, phase, Alu.mult, Alu.add,
                reads=["m1s"], writes=["sargA"])
            STT(t("cargA"), t("m1c"), -2 * PI, t("p2"), Alu.mult, Alu.add,
                reads=["m1c", "p2"], writes=["cargA"])
            STT(R("targ"), t("m2s"), 2 * PI, t("sargA"), Alu.mult, Alu.add,
                reads=["m2s", "sargA"], writes=["targ"])
            STT(L("targ"), t("m2c"), 2 * PI, t("cargA"), Alu.mult, Alu.add,
                reads=["m2c", "cargA"], writes=["targ"], inc=v1)

            # --- c/s-independent precompute (overlaps ACT) ---
            v.wait_ge(dma_b, 16)
            TT(t("sqp"), xy_pk, xy_pk, Alu.mult, writes=["sqp"])
            TS(t("negd"), xdo_pk[:, 0:F], -1.0, Alu.mult, writes=["negd"])
            TS(t("n1"), xdo_pk[:, 0:F], EPS, Alu.add, writes=["n1"])
            TT(t("r2"), L("sqp"), R("sqp"), Alu.add, reads=["sqp"],
               writes=["r2"])
            TT(t("d1"), t("negd"), xdo_pk[:, 0:F], Alu.max, reads=["negd"],
               writes=["d1"])
            q.op(lambda: v.memset(t("zeros"), 0.0), writes=["zeros"])
            TT(t("asq"), t("r2"), t("r2"), Alu.mult, reads=["r2"],
               writes=["asq"])
            TS(t("d1e"), t("d1"), EPS, Alu.add, reads=["d1"], writes=["d1e"])
            TS(t("a"), t("asq"), -ALPHA, Alu.mult, ALPHA, Alu.add,
               reads=["asq"], writes=["a"])
            q.op(lambda: v.reciprocal(t("rd"), t("d1e")), reads=["d1e"],
                 writes=["rd"])
            TT(t("t3"), t("a"), x, Alu.mult, reads=["a"], writes=["t3"])
            TT(t("ratio"), t("n1"), t("rd"), Alu.mult, reads=["n1", "rd"],
               writes=["ratio"])
            TT(t("t4"), t("a"), y, Alu.mult, reads=["a"], writes=["t4"])
            v.wait_ge(dma_w, 16)
            TT(t("hr"), ha, t("ratio"), Alu.mult, reads=["ratio"], writes=["hr"])
            TS(t("zeta"), t("hr"), -1.0, Alu.mult, 1.0 + EPS, Alu.add,
               reads=["hr"], writes=["zeta"])
            q.op(lambda: v.reciprocal(t("rz"), t("zeta")), reads=["zeta"],
                 writes=["rz"])
            TT(t("bt"), wfr, t("rz"), Alu.mult, reads=["rz"], writes=["bt"])
            TT(t("t5"), t("bt"), y, Alu.mult, reads=["bt"], writes=["t5"])
            TT(t("t6"), t("bt"), x, Alu.mult, reads=["bt"], writes=["t6"])
            TT(t("vx"), t("t3"), t("t5"), Alu.subtract, reads=["t3", "t5"],
               writes=["vx"])
            TT(t("vy"), t("t6"), t("t4"), Alu.add, reads=["t6", "t4"],
               writes=["vy"])

            # --- e = v - k*xy: iteration-0 dot basis.  The k*P@S0 term is
            # dropped from iteration 0 (its effect on the final output is
            # ~1e-8, below fp32) and S0 is instead folded into the carry
            # matmul, so iteration 0 has NO PE dependency at all. ---
            STT(L("qp"), x, -K_COUP, t("vx"), Alu.mult, Alu.add,
                reads=["vx"], writes=["qp"])
            STT(R("qp"), y, -K_COUP, t("vy"), Alu.mult, Alu.add,
                reads=["vy"], writes=["qp"])
            # dot0/z/scan chain, with the S0 column-sum ops (needed only by
            # the PE matmul) interleaved as the RAW-distance fillers
            v.wait_ge(a_s, 1)
            TT(t("dot"), t("qp"), t("lo"), Alu.max, reads=["qp"],
               writes=["dot"])
            TT(t("P1"), t("cs"), xy_pk, Alu.mult, writes=["P1"])
            TT(t("dot"), t("dot"), t("hi"), Alu.min, reads=["dot"],
               writes=["dot"])
            TS(R("P2"), x, -1.0, Alu.mult, writes=["P2"])
            q.op(lambda: v.tensor_copy(L("P2"), y), writes=["P2"])
            v.wait_ge(a_s, 2)
            TT(t("A"), t("dcs"), t("dot"), Alu.mult, reads=["dot"],
               writes=["A"])
            q.op(lambda: v.tensor_reduce(T["s0cols"][:, 0:1], t("P1"), AxX,
                                         Alu.add),
                 reads=["P1"], writes=["s0cols"])
            TT(t("B"), t("dsw"), t("dot"), Alu.mult, reads=["dot"],
               writes=["B"])
            TT(t("uAB"), t("cs"), t("P2"), Alu.mult, reads=["P2"],
               writes=["uAB"])
            TT(t("zx"), L("A"), R("A"), Alu.add, reads=["A"], writes=["zx"])
            q.op(lambda: v.tensor_reduce(T["s0cols"][:, 1:2], t("uAB"), AxX,
                                         Alu.add),
                 reads=["uAB"], writes=["s0cols"], inc=v2)
            TT(t("zy"), R("B"), L("B"), Alu.subtract, reads=["B"],
               writes=["zy"])
            # per-partition z totals via reduce (not the scan tails) so the
            # PE carry matmul overlaps the scans below
            q.op(lambda: v.tensor_reduce(T["lastc"][:, 0:1], t("zx"), AxX,
                                         Alu.add),
                 reads=["zx"], writes=["lastc"])
            q.op(lambda: v.tensor_reduce(T["lastc"][:, 1:2], t("zy"), AxX,
                                         Alu.add),
                 reads=["zy"], writes=["lastc"], inc=v3)
            q.op(lambda: v.tensor_tensor_scan(
                out=L("incl"), data0=t("zx"), data1=t("zeros"), initial=0.0,
                op0=Alu.add, op1=Alu.add),
                reads=["zx", "zeros"], writes=["incl"])
            q.op(lambda: v.tensor_tensor_scan(
                out=R("incl"), data0=t("zy"), data1=t("zeros"), initial=0.0,
                op0=Alu.add, op1=Alu.add),
                reads=["zy", "zeros"], writes=["incl"])

            # --- D+S0 = excl prefix + S0 (single accumulated PE psum) ---
            v.wait_ge(p_s, 1)
            STT(L("Dp"), L("incl"), cps[:, 0:1], t("zx"), Alu.add,
                Alu.subtract, reads=["incl", "zx"], writes=["Dp"])
            STT(R("Dp"), R("incl"), cps[:, 1:2], t("zy"), Alu.add,
                Alu.subtract, reads=["incl", "zy"], writes=["Dp"])
            TT(t("A"), t("kcs"), t("Dp"), Alu.mult, reads=["Dp"], writes=["A"])
            TT(t("B"), t("ksw"), t("Dp"), Alu.mult, reads=["Dp"], writes=["B"])
            TT(L("f"), L("A"), R("A"), Alu.subtract, reads=["A"], writes=["f"])
            TT(R("f"), L("B"), R("B"), Alu.add, reads=["B"], writes=["f"])
            TT(t("f"), t("f"), t("qp"), Alu.add, reads=["f", "qp"],
               writes=["f"])
            TT(t("dot"), t("f"), t("lo"), Alu.max, reads=["f"], writes=["dot"])
            TT(t("dot"), t("dot"), t("hi"), Alu.min, reads=["dot"],
               writes=["dot"])
            # angles = amp * (y + DT*doty) + b
            STT(t("ynew"), R("dot"), DT, y, Alu.mult, Alu.add,
                reads=["dot"], writes=["ynew"])
            TT(t("anga"), amp, t("ynew"), Alu.mult, reads=["ynew"],
               writes=["anga"])
            TT(t("ang"), t("anga"), bofs, Alu.add, reads=["anga"],
               writes=["ang"], inc=v_done)

    ctx.close()
    _strip_init_barrier(nc)
    return nc


def _strip_init_barrier(nc):
    """Remove the Bass-init all-engine rendezvous (4 Drains + EVSEM butterfly,
    ~3us) from the entry block.  Every ordering this kernel needs flows through
    its explicit semaphores: the Pool const-memsets precede the input DMA in
    Pool program order and all other engines gate on dma_s, so the rendezvous
    is redundant.  The Block-exit barrier is left untouched (removing it was
    observed to race)."""
    bb = nc.main_func.blocks[0]
    keep = [ins for ins in bb.instructions
            if not (type(ins).__name__ == "InstDrain"
                    or (type(ins).__name__ == "InstEventSemaphore"
                        and "barrier" in ins.name))]
    if len(keep) != len(bb.instructions):
        del bb.instructions[:]
        for ins in keep:
            bb.instructions.append(ins)


def _get_nc():
    if "nc" not in _CACHE:
        _CACHE["nc"] = _build()
    return _CACHE["nc"]


def pack_inputs(phase, amplitudes, w, ha, b, xy, xy_dot_old):
    f = np.float32
    xy = np.asarray(xy, f)
    xdo = np.asarray(xy_dot_old, f)
    planes = [
        np.asarray(phase, f).reshape(P, F),
        np.asarray(amplitudes, f).reshape(P, F),
        np.asarray(w, f).reshape(P, F),
        np.asarray(ha, f).reshape(P, F),
        np.asarray(b, f).reshape(P, F),
        np.ascontiguousarray(xy[:, 0]).reshape(P, F),
        np.ascontiguousarray(xy[:, 1]).reshape(P, F),
        np.ascontiguousarray(xdo[:, 0]).reshape(P, F),
        np.ascontiguousarray(xdo[:, 1]).reshape(P, F),
        np.triu(np.ones((P, P), f), k=1),
        np.ones((P, P), f),
    ]
    return {"inp": np.ascontiguousarray(np.concatenate(planes, axis=1))}


def kernel(phase, amplitudes, w, ha, b, xy, xy_dot_old, adj_mask):
    from concourse.bass_utils import run_bass_kernel_spmd

    nc = _get_nc()
    in_map = pack_inputs(phase, amplitudes, w, ha, b, xy, xy_dot_old)
    n_cores = 8
    res = run_bass_kernel_spmd(nc, [in_map] * n_cores, core_ids=list(range(n_cores)))
    return np.asarray(res.results[0]["angles"], dtype=np.float32).reshape(N)
